# revision 7
# baseline (speedup 1.0000x reference)
"""Trainium2 Bass kernel for a GAT-style attention head (B=2, N=6144, H=256, O=128).

Math (matching the reference):
  seq_fts = seq @ W_fts.T                       [B, N, O]
  f1 = seq_fts @ f1_w + f1_b                    [B, N]
  f2 = seq_fts @ f2_w + f2_b                    [B, N]
  z[b, j, i]  = leaky_relu(f1[b, i] + f2[b, j], 0.01)
  coefs[b,j,i] = softmax_b(z)   (B=2 -> coefs[0] = sigmoid(z0 - z1), coefs[1] = 1 - coefs[0])
  vals[b, i, o] = sum_j coefs[b, j, i] * seq_fts[b, j, o]
  out = elu(vals + bias)

Strategy: shard the output-row dim i across 8 cores (each core owns 768 rows).
Every core redundantly computes the full seq_fts (all j) plus f2; f1 only for
its own i-shard (via a separate per-core seq_shard input). The 2xNxN logits
are never materialized: a fused custom DVE op computes
d = lrelu(f1_0[i]+f2_0[j]) - lrelu(f1_1[i]+f2_1[j]) per [128j x 768i] tile,
ACT computes c0 = sigmoid(d), and the PE contracts c0 against
[fts0 | fts1] (float32r full-rate matmul). vals1 uses the complement trick:
vals1 = colsum(fts1) - c0 @ fts1.
"""

import numpy as np

import concourse.bacc as bacc
import concourse.bass as bass
import concourse.mybir as mybir
import concourse.tile as tile
from concourse.bass_utils import run_bass_kernel_spmd

B, N, H, O = 2, 6144, 256, 128
NCORES = 8
NS = N // NCORES          # 768 i-rows per core
NJT = N // 128            # 48 j-tiles
NIC = NS // 128           # 6 i-chunks per core
FP32 = mybir.dt.float32
F32R = mybir.dt.float32r
AF = mybir.ActivationFunctionType
ALU = mybir.AluOpType

_DVE_OP_NAME = "DIFF_LRELU_ANT"


def _get_diff_lrelu_op():
    """Register (once) and return the fused custom DVE op:
    out = lrelu(in0 + s0) - lrelu(in1 + s1), slope imm2."""
    import concourse.dve_ops as dve_ops
    from concourse.dve_ops import OPS, DveOp

    for op in OPS:
        if op.name == _DVE_OP_NAME:
            return op

    from concourse.dve_spec import C0, C1, C2, Spec, Src0, Src1, lower, maxx
    from concourse.dve_uop import DveOpSpec

    a = Src0 + C0
    b = Src1 + C1
    spec = Spec(
        body=maxx(a, a * C2) - maxx(b, b * C2),
        reference=lambda in0, in1, s0, s1, imm2: (
            np.maximum(in0 + s0, (in0 + s0) * imm2)
            - np.maximum(in1 + s1, (in1 + s1) * imm2)
        ).astype(np.float32),
    )
    row = dve_ops._CUSTOM_DVE_ROW_BASE + len(OPS)
    shas = {}
    for ver in ("v3",):
        uops = lower(spec, ver=ver)
        shas[ver] = DveOpSpec(
            name=_DVE_OP_NAME, opcode=row, uops=uops, rd1_en=True
        ).sha(ver)
    op = DveOp(_DVE_OP_NAME, spec, subdim=False, uops_sha=shas)
    OPS.append(op)
    dve_ops.CUSTOM_DVE_SPECS[_DVE_OP_NAME] = spec
    dve_ops._SUB_OPCODE_FOR_NAME[_DVE_OP_NAME] = row
    return op


def build_nc(probes=False):
    diff_lrelu = _get_diff_lrelu_op()

    nc = bacc.Bacc("TRN2", target_bir_lowering=False, debug=False, num_devices=NCORES)

    seq_d = nc.declare_dram_parameter("seq", [B, N, H], FP32, isOutput=False)
    seqs_d = nc.declare_dram_parameter("seq_shard", [B, NS, H], FP32, isOutput=False)
    wtg_d = nc.declare_dram_parameter("wtg", [2, 128, 256], FP32, isOutput=False)
    g1r_d = nc.declare_dram_parameter("g1r", [1, H], FP32, isOutput=False)
    consts_d = nc.declare_dram_parameter("consts", [1, 4], FP32, isOutput=False)
    ident_d = nc.declare_dram_parameter("ident", [128, 128], FP32, isOutput=False)
    out_d = nc.declare_dram_parameter("out", [B, NS, O], FP32, isOutput=True)
    if probes:
        pr_f1 = nc.declare_dram_parameter("pr_f1", [B, NS], FP32, isOutput=True)
        pr_f2 = nc.declare_dram_parameter("pr_f2", [B, NJT, 128], FP32, isOutput=True)
        pr_fts = nc.declare_dram_parameter("pr_fts", [B, NJT, 128, O], FP32, isOutput=True)
        pr_d = nc.declare_dram_parameter("pr_d", [128, NS], FP32, isOutput=True)
        pr_c0 = nc.declare_dram_parameter("pr_c0", [128, NS], FP32, isOutput=True)
        pr_s1 = nc.declare_dram_parameter("pr_s1", [1, O], FP32, isOutput=True)
        pr_tred = nc.declare_dram_parameter("pr_tred", [128, 2], FP32, isOutput=True)
        pr_vals = nc.declare_dram_parameter("pr_vals", [128, B, O], FP32, isOutput=True)

    with tile.TileContext(nc) as tc:
        with (
            tc.tile_pool(name="const", bufs=1) as cpool,
            tc.tile_pool(name="seq_in", bufs=4) as p_seq,
            tc.tile_pool(name="sT", bufs=3) as p_sT,
            tc.tile_pool(name="dtile", bufs=3) as p_d,
            tc.tile_pool(name="ctile", bufs=3) as p_c,
            tc.tile_pool(name="fin", bufs=8) as p_fin,
            tc.tile_pool(name="outt", bufs=4) as p_out,
        ):
            # ---------------- constants / setup ----------------
            wtg = cpool.tile([128, 2, 256], FP32)
            nc.sync.dma_start(wtg[:], wtg_d.ap().rearrange("k p c -> p k c"))
            wtg_r = cpool.tile([128, 2, 256], F32R)
            nc.vector.tensor_copy(wtg_r[:], wtg[:])
            ident = cpool.tile([128, 128], FP32)
            nc.sync.dma_start(ident[:], ident_d[:])
            consts = cpool.tile([1, 4], FP32)
            nc.sync.dma_start(consts[:], consts_d[:])
            g1row = cpool.tile([1, H], FP32)
            nc.sync.dma_start(g1row[:], g1r_d[:])

            bias_col = cpool.tile([128, 1], FP32)
            nc.gpsimd.partition_broadcast(bias_col[:], consts[0:1, 1:2])
            biasm1_col = cpool.tile([128, 1], FP32)
            nc.gpsimd.partition_broadcast(biasm1_col[:], consts[0:1, 2:3])
            g1bc = cpool.tile([128, H], FP32)
            nc.gpsimd.partition_broadcast(g1bc[:], g1row[:])

            fts = cpool.tile([128, NJT, B, 130], F32R)     # [n128, jt, b, {fts[0:128], f1, f2}]
            tpart = cpool.tile([128, 2, NJT], FP32)        # b=1 column-sum partials of seq^T
            f1row = [cpool.tile([1, NS], FP32, name=f"f1row{b}") for b in range(B)]
            f1bc = [cpool.tile([128, NS], FP32, name=f"f1bc{b}") for b in range(B)]
            s1bc = cpool.tile([128, O], FP32)
            s1row = cpool.tile([1, O], FP32)

            # ---------------- shard pass: f1 for own i-rows ----------------
            with tc.tile_pool(name="ps_shard", bufs=2, space="PSUM") as ps_sh:
                for b in range(B):
                    for nt in range(NIC):
                        ss = p_seq.tile([128, H], FP32)
                        nc.sync.dma_start(ss[:], seqs_d[b, nt * 128:(nt + 1) * 128, :])
                        prod = p_sT.tile([128, H], FP32, tag="shard_prod")
                        nc.vector.tensor_tensor(prod[:], ss[:], g1bc[:], ALU.mult)
                        f1c = p_fin.tile([128, 1], FP32, tag="shard_f1c")
                        nc.vector.tensor_reduce(f1c[:], prod[:], mybir.AxisListType.X, ALU.add)
                        f1p = ps_sh.tile([1, 128], FP32)
                        nc.tensor.transpose(f1p[:], f1c[:], ident[:])
                        nc.vector.tensor_copy(f1row[b][0:1, nt * 128:(nt + 1) * 128], f1p[:])
                for b in range(B):
                    # += (f1_b + f2_b); the f2 column stays raw.
                    nc.vector.tensor_scalar(
                        f1row[b][:], f1row[b][:], consts[0:1, 0:1], None, ALU.add
                    )
                    nc.gpsimd.partition_broadcast(f1bc[b][:], f1row[b][:])

            # ---------------- stage A: seq_fts / f1 / f2 for all j ----------------
            with (
                tc.tile_pool(name="psA", bufs=4, space="PSUM") as psA,
                tc.tile_pool(name="psF", bufs=2, space="PSUM") as psF,
            ):
                for jt in range(NJT):
                    for b in range(B):
                        s = p_seq.tile([128, H], FP32)
                        nc.sync.dma_start(s[:], seq_d[b, jt * 128:(jt + 1) * 128, :])
                        sT = p_sT.tile([128, 2, 128], F32R)
                        for kt in range(2):
                            pt = psA.tile([128, 128], FP32)
                            nc.tensor.transpose(pt[:], s[:, kt * 128:(kt + 1) * 128], ident[:])
                            if b == 1:
                                nc.scalar.activation(
                                    sT[:, kt, :], pt[:], AF.Copy,
                                    accum_out=tpart[:, kt, jt:jt + 1],
                                )
                            elif kt == 0:
                                nc.vector.tensor_copy(sT[:, kt, :], pt[:])
                            else:
                                nc.scalar.activation(sT[:, kt, :], pt[:], AF.Copy)
                        fp = psF.tile([128, 256], FP32)
                        for kt in range(2):
                            nc.tensor.matmul(
                                fp[:],
                                lhsT=sT[:, kt, :],
                                rhs=wtg_r[:, kt, :],
                                start=(kt == 0),
                                stop=(kt == 1),
                            )
                        if b == 0:
                            nc.vector.tensor_copy(fts[:, jt, b, :], fp[:, 0:130])
                        else:
                            nc.scalar.activation(fts[:, jt, b, :], fp[:, 0:130], AF.Copy)

                # S1[o] = sum_j seq_fts[1, j, o] = (sum_n seq[1, n, :]) @ WT
                tred = cpool.tile([128, 2], FP32)
                nc.vector.tensor_reduce(tred[:], tpart[:], mybir.AxisListType.X, ALU.add)
                s1p = psF.tile([1, 256], FP32)
                for kt in range(2):
                    nc.tensor.matmul(
                        s1p[:],
                        lhsT=tred[:, kt:kt + 1],
                        rhs=wtg[:, kt, :],
                        start=(kt == 0),
                        stop=(kt == 1),
                    )
                nc.vector.tensor_copy(s1row[:], s1p[0:1, 0:O])
                nc.gpsimd.partition_broadcast(s1bc[:], s1row[:])
                if probes:
                    nc.sync.dma_start(pr_s1[:], s1row[:])
                    nc.sync.dma_start(pr_tred[:], tred[:])
                    for b in range(B):
                        nc.sync.dma_start(pr_f1[b:b+1, :], f1row[b][:])
                        for jt in range(NJT):
                            nc.sync.dma_start(
                                pr_f2[b, jt, :],
                                fts[:, jt, b, 129:130].bitcast(FP32),
                            )
                            nc.sync.dma_start(
                                pr_fts[b, jt, :, :],
                                fts[:, jt, b, 0:O].bitcast(FP32),
                            )

            # ---------------- stage B: fused scores + matmul ----------------
            with tc.tile_pool(name="psB", bufs=1, space="PSUM") as psB:
                pacc = [
                    psB.tile([128, B, O], FP32, name=f"pacc{ic}", tag=f"pacc{ic}")
                    for ic in range(NIC)
                ]
                for jt in range(NJT):
                    d = p_d.tile([128, NS], FP32)
                    nc.vector._custom_dve(
                        diff_lrelu,
                        out=d[:],
                        in0=f1bc[0][:],
                        in1=f1bc[1][:],
                        s0=fts[:, jt, 0, 129:130].bitcast(FP32),
                        s1=fts[:, jt, 1, 129:130].bitcast(FP32),
                        imm2=0.01,
                    )
                    c0 = p_c.tile([128, NS], F32R)
                    nc.scalar.activation(c0[:], d[:], AF.Sigmoid)
                    if probes and jt == 0:
                        nc.sync.dma_start(pr_d[:], d[:])
                        nc.sync.dma_start(pr_c0[:], c0[:].bitcast(FP32))
                    for ic in range(NIC):
                        nc.tensor.matmul(
                            pacc[ic][:],
                            lhsT=c0[:, ic * 128:(ic + 1) * 128],
                            rhs=fts[:, jt, :, 0:O],
                            start=(jt == 0),
                            stop=(jt == NJT - 1),
                        )

                if probes:
                    pv = p_c.tile([128, B * O], FP32, name="pv", tag="pv")
                    nc.vector.tensor_copy(pv[:], pacc[0][:])
                    nc.sync.dma_start(pr_vals.ap().rearrange("p b o -> p (b o)"), pv[:])

                # ---------------- finalize: elu(vals + bias) ----------------
                def elu_store(src_ap, b, ic):
                    # elu(y) with y = src + bias: relu(y)-1 = max(y-1,-1); + exp(min(y,0))
                    r = p_fin.tile([128, O], FP32, tag="fin_r")
                    nc.vector.tensor_scalar(
                        r[:], src_ap, biasm1_col[:], -1.0, ALU.add, ALU.max
                    )
                    m = p_fin.tile([128, O], FP32, tag="fin_m")
                    nc.vector.tensor_scalar(
                        m[:], src_ap, bias_col[:], 0.0, ALU.add, ALU.min
                    )
                    e = p_fin.tile([128, O], FP32, tag="fin_e")
                    nc.scalar.activation(e[:], m[:], AF.Exp)
                    o = p_out.tile([128, O], FP32)
                    nc.vector.tensor_tensor(o[:], r[:], e[:], ALU.add)
                    nc.sync.dma_start(out_d[b, ic * 128:(ic + 1) * 128, :], o[:])

                for ic in range(NIC):
                    elu_store(pacc[ic][:, 0, :], 0, ic)
                    x1 = p_fin.tile([128, O], FP32, tag="fin_x1")
                    nc.vector.tensor_tensor(x1[:], s1bc[:], pacc[ic][:, 1, :], ALU.subtract)
                    elu_store(x1[:], 1, ic)

    nc.compile()
    return nc


def make_in_maps(seq, W_fts, f1_w, f1_b, f2_w, f2_b, bias):
    seq = np.ascontiguousarray(np.asarray(seq, dtype=np.float32))
    W = np.asarray(W_fts, dtype=np.float32)
    f1_w = np.asarray(f1_w, dtype=np.float32).reshape(-1)
    f2_w = np.asarray(f2_w, dtype=np.float32).reshape(-1)
    WT = np.ascontiguousarray(W.T)                      # [H, O]
    g1 = WT @ f1_w                                      # [H]
    g2 = WT @ f2_w
    wtg = np.zeros((2, 128, 256), np.float32)
    for kt in range(2):
        wtg[kt, :, 0:O] = WT[kt * 128:(kt + 1) * 128]
        wtg[kt, :, O] = g1[kt * 128:(kt + 1) * 128]
        wtg[kt, :, O + 1] = g2[kt * 128:(kt + 1) * 128]
    fsum = float(np.asarray(f1_b).reshape(-1)[0] + np.asarray(f2_b).reshape(-1)[0])
    bs = float(np.asarray(bias).reshape(-1)[0])
    consts = np.array([[fsum, bs, bs - 1.0, 0.0]], np.float32)
    ident = np.eye(128, dtype=np.float32)
    g1r = g1.reshape(1, H)

    in_maps = []
    for c in range(NCORES):
        in_maps.append({
            "seq": seq,
            "seq_shard": np.ascontiguousarray(seq[:, c * NS:(c + 1) * NS, :]),
            "wtg": wtg,
            "g1r": g1r,
            "consts": consts,
            "ident": ident,
        })
    return in_maps


_NC_CACHE = []


def kernel(seq, W_fts, f1_w, f1_b, f2_w, f2_b, bias):
    if not _NC_CACHE:
        _NC_CACHE.append(build_nc())
    nc = _NC_CACHE[0]
    in_maps = make_in_maps(seq, W_fts, f1_w, f1_b, f2_w, f2_b, bias)
    res = run_bass_kernel_spmd(nc, in_maps, core_ids=list(range(NCORES)))
    return np.concatenate([res.results[c]["out"] for c in range(NCORES)], axis=1)


# revision 23
# speedup vs baseline: 1.2289x; 1.2289x over previous
"""Trainium2 Bass kernel for a GAT-style attention head (B=2, N=6144, H=256, O=128).

Math (matching the reference):
  seq_fts = seq @ W_fts.T                       [B, N, O]
  f1 = seq_fts @ f1_w + f1_b                    [B, N]
  f2 = seq_fts @ f2_w + f2_b                    [B, N]
  z[b, j, i]  = leaky_relu(f1[b, i] + f2[b, j], 0.01)
  coefs[b,j,i] = softmax_b(z)   (B=2 -> coefs[0] = sigmoid(z0 - z1), coefs[1] = 1 - coefs[0])
  vals[b, i, o] = sum_j coefs[b, j, i] * seq_fts[b, j, o]
  out = elu(vals + bias)

Strategy: shard the output-row dim i across 8 cores (each core owns 768 rows).
Every core redundantly computes the full seq_fts (all j) plus f2; f1 only for
its own i-shard (via a separate per-core seq_shard input). The 2xNxN logits
are never materialized: a fused custom DVE op computes
d = lrelu(f1_0[i]+f2_0[j]) - lrelu(f1_1[i]+f2_1[j]) per [128j x 768i] tile,
ACT computes c0 = sigmoid(d), and the PE contracts c0 against
[fts0 | fts1] (float32r full-rate matmul). vals1 uses the complement trick:
vals1 = colsum(fts1) - c0 @ fts1.
"""

import numpy as np

import concourse.bacc as bacc
import concourse.bass as bass
import concourse.mybir as mybir
import concourse.tile as tile
from concourse.bass_utils import run_bass_kernel_spmd

B, N, H, O = 2, 6144, 256, 128
NCORES = 8
NS = N // NCORES          # 768 i-rows per core
NJT = N // 128            # 48 j-tiles
NIC = NS // 128           # 6 i-chunks per core
FP32 = mybir.dt.float32
BF16 = mybir.dt.bfloat16
F32R = mybir.dt.float32r
AF = mybir.ActivationFunctionType
ALU = mybir.AluOpType

_DVE_OP_NAME = "DIFF_LRELU_ANT"

DEFAULT_CFG = dict(
    lag=16,             # stage B runs `lag` j-tiles behind stage A (NJT = serial)
    bufs_seq=6,
    bufs_sT=4,
    bufs_d=5,
    bufs_c=5,
    bufs_psA=2,
    bufs_psF=2,
    fts_copy="dve",     # engine for psum->fts copies: dve | act | alt
    stage_a="dmat",     # dmat (bf16 DMA transpose) | pe (fp32 PE transpose)
    sT_b0_copy="dve",   # legacy, unused
    sT_copy="act",      # engine for the merged transpose-block copy: dve | act | split
    ablate_scores=False,    # stage B matmuls use a constant lhsT (skip d/sigmoid)
    ablate_transposes=False,  # skip PE transposes + copies (sT from dummy)
    ablate_dma=False,       # load only one seq tile
)


def _get_diff_lrelu_op():
    """Register (once) and return the fused custom DVE op:
    out = lrelu(in0 + s0) - lrelu(in1 + s1), slope imm2."""
    import concourse.dve_ops as dve_ops
    from concourse.dve_ops import OPS, DveOp

    for op in OPS:
        if op.name == _DVE_OP_NAME:
            return op

    from concourse.dve_spec import C0, C1, C2, Spec, Src0, Src1, lower, maxx
    from concourse.dve_uop import DveOpSpec

    a = Src0 + C0
    b = Src1 + C1
    spec = Spec(
        body=maxx(a, a * C2) - maxx(b, b * C2),
        reference=lambda in0, in1, s0, s1, imm2: (
            np.maximum(in0 + s0, (in0 + s0) * imm2)
            - np.maximum(in1 + s1, (in1 + s1) * imm2)
        ).astype(np.float32),
    )
    row = dve_ops._CUSTOM_DVE_ROW_BASE + len(OPS)
    shas = {}
    for ver in ("v3",):
        uops = lower(spec, ver=ver)
        shas[ver] = DveOpSpec(
            name=_DVE_OP_NAME, opcode=row, uops=uops, rd1_en=True
        ).sha(ver)
    op = DveOp(_DVE_OP_NAME, spec, subdim=False, uops_sha=shas)
    OPS.append(op)
    dve_ops.CUSTOM_DVE_SPECS[_DVE_OP_NAME] = spec
    dve_ops._SUB_OPCODE_FOR_NAME[_DVE_OP_NAME] = row
    return op


def build_nc(probes=False, cfg=None):
    cfg = {**DEFAULT_CFG, **(cfg or {})}
    diff_lrelu = _get_diff_lrelu_op()

    nc = bacc.Bacc("TRN2", target_bir_lowering=False, debug=False, num_devices=NCORES)

    seq_d = nc.declare_dram_parameter("seq", [B, N, H], FP32, isOutput=False)
    seqs_d = nc.declare_dram_parameter("seq_shard", [B, NS, H], FP32, isOutput=False)
    wtg_d = nc.declare_dram_parameter("wtg", [2, 128, 256], FP32, isOutput=False)
    g1r_d = nc.declare_dram_parameter("g1r", [1, H], FP32, isOutput=False)
    consts_d = nc.declare_dram_parameter("consts", [1, 4], FP32, isOutput=False)
    ident_d = nc.declare_dram_parameter("ident", [128, 128], FP32, isOutput=False)
    out_d = nc.declare_dram_parameter("out", [B, NS, O], FP32, isOutput=True)
    if probes:
        pr_f1 = nc.declare_dram_parameter("pr_f1", [B, NS], FP32, isOutput=True)
        pr_f2 = nc.declare_dram_parameter("pr_f2", [B, NJT, 128], FP32, isOutput=True)
        pr_fts = nc.declare_dram_parameter("pr_fts", [B, NJT, 128, O], FP32, isOutput=True)
        pr_d = nc.declare_dram_parameter("pr_d", [128, NS], FP32, isOutput=True)
        pr_c0 = nc.declare_dram_parameter("pr_c0", [128, NS], FP32, isOutput=True)
        pr_s1 = nc.declare_dram_parameter("pr_s1", [1, O], FP32, isOutput=True)
        pr_tred = nc.declare_dram_parameter("pr_tred", [128, 2], FP32, isOutput=True)
        pr_vals = nc.declare_dram_parameter("pr_vals", [128, B, O], FP32, isOutput=True)

    with tile.TileContext(nc) as tc:
        with (
            tc.tile_pool(name="const", bufs=1) as cpool,
            tc.tile_pool(name="seq_in", bufs=cfg["bufs_seq"]) as p_seq,
            tc.tile_pool(name="sT", bufs=cfg["bufs_sT"]) as p_sT,
            tc.tile_pool(name="dtile", bufs=cfg["bufs_d"]) as p_d,
            tc.tile_pool(name="ctile", bufs=cfg["bufs_c"]) as p_c,
            tc.tile_pool(name="fin", bufs=8) as p_fin,
            tc.tile_pool(name="outt", bufs=4) as p_out,
        ):
            # ---------------- constants / setup ----------------
            wtg = cpool.tile([128, 2, 256], FP32)
            nc.sync.dma_start(wtg[:], wtg_d.ap().rearrange("k p c -> p k c"))
            wtg_r = cpool.tile([128, 2, 256], F32R)
            nc.vector.tensor_copy(wtg_r[:], wtg[:])
            wtg_bf = cpool.tile([128, 2, 256], BF16)
            nc.vector.tensor_copy(wtg_bf[:], wtg[:])
            ones_f = cpool.tile([128, 1], FP32)
            nc.gpsimd.memset(ones_f[:], 1.0)
            ones_r = cpool.tile([128, 1], F32R)
            nc.vector.tensor_copy(ones_r[:], ones_f[:])
            ident = cpool.tile([128, 128], FP32)
            nc.sync.dma_start(ident[:], ident_d[:])
            consts = cpool.tile([1, 4], FP32)
            nc.sync.dma_start(consts[:], consts_d[:])
            g1row = cpool.tile([1, H], FP32)
            nc.sync.dma_start(g1row[:], g1r_d[:])

            bias_col = cpool.tile([128, 1], FP32)
            nc.gpsimd.partition_broadcast(bias_col[:], consts[0:1, 1:2])
            biasm1_col = cpool.tile([128, 1], FP32)
            nc.gpsimd.partition_broadcast(biasm1_col[:], consts[0:1, 2:3])
            g1bc = cpool.tile([128, H], FP32)
            nc.gpsimd.partition_broadcast(g1bc[:], g1row[:])

            fts = cpool.tile([128, NJT, B, 130], F32R)     # [n128, jt, b, {fts[0:128], f1, f2}]
            f1row = [cpool.tile([1, NS], FP32, name=f"f1row{b}") for b in range(B)]
            f1bc = [cpool.tile([128, NS], FP32, name=f"f1bc{b}") for b in range(B)]
            s1bc = cpool.tile([128, O], FP32)
            s1row = cpool.tile([1, O], FP32)

            # shard pass pieces (f1 for own i-rows) - emitted interleaved below
            def shard_piece(k, ps_sh):
                del ps_sh
                b, nt = divmod(k, NIC)
                ss = p_seq.tile([128, H], FP32, name="ss", tag="s")
                nc.sync.dma_start(ss[:], seqs_d[b, nt * 128:(nt + 1) * 128, :])
                prod = p_sT.tile([128, H], FP32, tag="shard_prod")
                nc.vector.tensor_tensor(prod[:], ss[:], g1bc[:], ALU.mult)
                f1c = p_fin.tile([128, 1], FP32, tag="shard_f1c")
                nc.vector.tensor_reduce(f1c[:], prod[:], mybir.AxisListType.X, ALU.add)
                f1p = psA.tile([1, 128], FP32, name="f1p", tag="pt")
                nc.tensor.transpose(f1p[:], f1c[:], ident[:])
                nc.vector.tensor_copy(f1row[b][0:1, nt * 128:(nt + 1) * 128], f1p[:])

            def shard_finish():
                for b in range(B):
                    # += (f1_b + f2_b); the f2 column stays raw.
                    nc.vector.tensor_scalar(
                        f1row[b][:], f1row[b][:], consts[0:1, 0:1], None, ALU.add
                    )
                    nc.gpsimd.partition_broadcast(f1bc[b][:], f1row[b][:])

            # ------- stage A + stage B software-pipelined over j-tiles -------
            with (
                tc.tile_pool(name="psA", bufs=cfg["bufs_psA"], space="PSUM") as psA,
                tc.tile_pool(name="psF", bufs=cfg["bufs_psF"], space="PSUM") as psF,
                tc.tile_pool(name="psS", bufs=1, space="PSUM") as psS,
                tc.tile_pool(name="psB", bufs=1, space="PSUM") as psB,
            ):
                s1acc = psS.tile([1, B, O], FP32, name="s1acc", tag="s1acc")
                # 6 accumulators packed 2-per-bank: pacc_ap(ic) is one i-chunk
                pacc2 = [
                    psB.tile([128, 2, B, O], FP32, name=f"pacc{k}", tag=f"pacc{k}")
                    for k in range(NIC // 2)
                ]

                def pacc_ap(ic):
                    return pacc2[ic // 2][:, ic % 2]

                dummy_c = cpool.tile([128, NS], F32R)
                nc.vector.tensor_copy(dummy_c[:, 0:256], wtg_r[:, 0, :])
                if cfg["ablate_transposes"] or cfg["ablate_dma"]:
                    s0_tile = p_seq.tile([128, H], FP32, name="s", tag="s")
                    nc.sync.dma_start(s0_tile[:], seq_d[0, 0:128, :])
                    sT0 = p_sT.tile([128, 2, 128], F32R, name="sT0", tag="sT0")
                    for kt in range(2):
                        nc.vector.tensor_copy(sT0[:, kt, :], s0_tile[:, 0:128])
                if cfg["ablate_transposes"]:
                    nc.gpsimd.memset(tpart[:], 0.0)

                sT_tiles = {}

                def stage_t(jt):
                    # DMA in + PE transposes for j-tile jt
                    ss = []
                    for b in range(B):
                        st = p_seq.tile([128, H], FP32, name="s", tag="s")
                        nc.sync.dma_start(st[:], seq_d[b, jt * 128:(jt + 1) * 128, :])
                        ss.append(st)
                    pt = psA.tile([128, 4, 128], FP32, name="pt", tag="pt")
                    sT = p_sT.tile([128, 4, 128], F32R, name="sT", tag="sT")
                    for b in range(B):
                        for kt in range(2):
                            nc.tensor.transpose(
                                pt[:, b * 2 + kt],
                                ss[b][:, kt * 128:(kt + 1) * 128], ident[:],
                            )
                    if cfg["sT_copy"] == "dve":
                        nc.vector.tensor_copy(sT[:], pt[:])
                    elif cfg["sT_copy"] == "act":
                        nc.scalar.activation(sT[:], pt[:], AF.Copy)
                    else:
                        nc.vector.tensor_copy(sT[:, 0:2], pt[:, 0:2])
                        nc.scalar.activation(sT[:, 2:4], pt[:, 2:4], AF.Copy)
                    sT_tiles[jt] = sT

                def stage_m(jt):
                    # projection matmuls + psum->sbuf copy for j-tile jt
                    sT = sT_tiles.pop(jt)
                    fpp = psF.tile([128, 2, 256], FP32, name="fpp", tag="fpp")
                    for b in range(B):
                        for kt in range(2):
                            nc.tensor.matmul(
                                fpp[:, b],
                                lhsT=sT[:, b * 2 + kt, :],
                                rhs=wtg_r[:, kt, :],
                                start=(kt == 0), stop=(kt == 1),
                            )
                    eng = cfg["fts_copy"]
                    if eng == "alt":
                        eng = "dve" if jt % 2 == 0 else "act"
                    if eng == "dve":
                        nc.vector.tensor_copy(fts[:, jt, :, :], fpp[:, :, 0:130])
                    else:
                        nc.scalar.activation(fts[:, jt, :, :], fpp[:, :, 0:130], AF.Copy)

                def stage_b(jt):
                    if cfg["ablate_scores"]:
                        for ic in range(NIC):
                            nc.tensor.matmul(
                                pacc_ap(ic),
                                lhsT=dummy_c[:, ic * 128:(ic + 1) * 128],
                                rhs=fts[:, jt, :, 0:O],
                                start=(jt == 0),
                                stop=(jt == NJT - 1),
                            )
                        return
                    d = p_d.tile([128, NS], FP32, name="d", tag="d")
                    nc.vector._custom_dve(
                        diff_lrelu,
                        out=d[:],
                        in0=f1bc[0][:],
                        in1=f1bc[1][:],
                        s0=fts[:, jt, 0, 129:130].bitcast(FP32),
                        s1=fts[:, jt, 1, 129:130].bitcast(FP32),
                        imm2=0.01,
                    )
                    c0 = p_c.tile([128, NS], F32R, name="c0", tag="c0")
                    nc.scalar.activation(c0[:], d[:], AF.Sigmoid)
                    if probes and jt == 0:
                        nc.sync.dma_start(pr_d[:], d[:])
                        nc.sync.dma_start(pr_c0[:], c0[:].bitcast(FP32))
                    for ic in range(NIC):
                        # start=True clears the WHOLE psum bank, so only the
                        # first chunk sharing a bank may issue it; the second
                        # chunk's first write lands on cleared has_written
                        # bits and overwrites cleanly with start=False.
                        nc.tensor.matmul(
                            pacc_ap(ic),
                            lhsT=c0[:, ic * 128:(ic + 1) * 128],
                            rhs=fts[:, jt, :, 0:O],
                            start=(jt == 0 and ic % 2 == 0),
                            stop=(jt == NJT - 1),
                            skip_group_check=True,
                        )
                    nc.tensor.matmul(
                        s1acc[:], lhsT=ones_r[:], rhs=fts[:, jt, :, 0:O],
                        start=(jt == 0), stop=(jt == NJT - 1),
                    )

                lag = max(2, min(cfg["lag"], NJT))
                if True:
                    ps_sh = None
                    for jt in range(NJT + lag):
                        if jt < NJT:
                            stage_t(jt)
                        if jt < 2 * NIC:
                            shard_piece(jt, ps_sh)
                        elif jt == 2 * NIC:
                            shard_finish()
                        if jt >= 1 and jt - 1 < NJT:
                            stage_m(jt - 1)
                        if jt >= lag:
                            stage_b(jt - lag)

                nc.vector.tensor_copy(s1row[:], s1acc[0:1, 1, :])
                nc.gpsimd.partition_broadcast(s1bc[:], s1row[:])
                if probes:
                    nc.sync.dma_start(pr_s1[:], s1row[:])
                    for b in range(B):
                        nc.sync.dma_start(pr_f1[b:b + 1, :], f1row[b][:])
                        for jt in range(NJT):
                            nc.sync.dma_start(
                                pr_f2[b, jt, :],
                                fts[:, jt, b, 129:130].bitcast(FP32),
                            )
                            nc.sync.dma_start(
                                pr_fts[b, jt, :, :],
                                fts[:, jt, b, 0:O].bitcast(FP32),
                            )
                    pv = p_c.tile([128, B * O], FP32, name="pv", tag="pv")
                    nc.vector.tensor_copy(pv[:], pacc_ap(0))
                    nc.sync.dma_start(pr_vals.ap().rearrange("p b o -> p (b o)"), pv[:])

                # ---------------- finalize: elu(vals + bias) ----------------
                def elu_store(src_ap, b, ic):
                    # elu(y) with y = src + bias: relu(y)-1 = max(y-1,-1); + exp(min(y,0))
                    r = p_fin.tile([128, O], FP32, tag="fin_r")
                    nc.vector.tensor_scalar(
                        r[:], src_ap, biasm1_col[:], -1.0, ALU.add, ALU.max
                    )
                    m = p_fin.tile([128, O], FP32, tag="fin_m")
                    nc.vector.tensor_scalar(
                        m[:], src_ap, bias_col[:], 0.0, ALU.add, ALU.min
                    )
                    e = p_fin.tile([128, O], FP32, tag="fin_e")
                    nc.scalar.activation(e[:], m[:], AF.Exp)
                    o = p_out.tile([128, O], FP32)
                    nc.vector.tensor_tensor(o[:], r[:], e[:], ALU.add)
                    nc.sync.dma_start(out_d[b, ic * 128:(ic + 1) * 128, :], o[:])

                for ic in range(NIC):
                    elu_store(pacc_ap(ic)[:, 0, :], 0, ic)
                    x1 = p_fin.tile([128, O], FP32, tag="fin_x1")
                    nc.vector.tensor_tensor(x1[:], s1bc[:], pacc_ap(ic)[:, 1, :], ALU.subtract)
                    elu_store(x1[:], 1, ic)

    nc.compile()
    return nc


def make_in_maps(seq, W_fts, f1_w, f1_b, f2_w, f2_b, bias):
    seq = np.ascontiguousarray(np.asarray(seq, dtype=np.float32))
    W = np.asarray(W_fts, dtype=np.float32)
    f1_w = np.asarray(f1_w, dtype=np.float32).reshape(-1)
    f2_w = np.asarray(f2_w, dtype=np.float32).reshape(-1)
    WT = np.ascontiguousarray(W.T)                      # [H, O]
    g1 = WT @ f1_w                                      # [H]
    g2 = WT @ f2_w
    wtg = np.zeros((2, 128, 256), np.float32)
    for kt in range(2):
        wtg[kt, :, 0:O] = WT[kt * 128:(kt + 1) * 128]
        wtg[kt, :, O] = g1[kt * 128:(kt + 1) * 128]
        wtg[kt, :, O + 1] = g2[kt * 128:(kt + 1) * 128]
    fsum = float(np.asarray(f1_b).reshape(-1)[0] + np.asarray(f2_b).reshape(-1)[0])
    bs = float(np.asarray(bias).reshape(-1)[0])
    consts = np.array([[fsum, bs, bs - 1.0, 0.0]], np.float32)
    ident = np.eye(128, dtype=np.float32)
    g1r = g1.reshape(1, H)

    in_maps = []
    for c in range(NCORES):
        in_maps.append({
            "seq": seq,
            "seq_shard": np.ascontiguousarray(seq[:, c * NS:(c + 1) * NS, :]),
            "wtg": wtg,
            "g1r": g1r,
            "consts": consts,
            "ident": ident,
        })
    return in_maps


_NC_CACHE = []


def kernel(seq, W_fts, f1_w, f1_b, f2_w, f2_b, bias):
    if not _NC_CACHE:
        _NC_CACHE.append(build_nc())
    nc = _NC_CACHE[0]
    in_maps = make_in_maps(seq, W_fts, f1_w, f1_b, f2_w, f2_b, bias)
    res = run_bass_kernel_spmd(nc, in_maps, core_ids=list(range(NCORES)))
    return np.concatenate([res.results[c]["out"] for c in range(NCORES)], axis=1)


# revision 33
# speedup vs baseline: 907.4617x; 738.4478x over previous
"""Trainium2 Bass kernel for a GAT-style attention head (B=2, N=6144, H=256, O=128).

Math (matching the reference):
  seq_fts = seq @ W_fts.T                       [B, N, O]
  f1 = seq_fts @ f1_w + f1_b                    [B, N]
  f2 = seq_fts @ f2_w + f2_b                    [B, N]
  z[b, j, i]  = leaky_relu(f1[b, i] + f2[b, j], 0.01)
  coefs[b,j,i] = softmax_b(z)   (B=2 -> coefs[0] = sigmoid(z0 - z1), coefs[1] = 1 - coefs[0])
  vals[b, i, o] = sum_j coefs[b, j, i] * seq_fts[b, j, o]
  out = elu(vals + bias)

Strategy: shard the output-row dim i across 8 cores (each core owns 768 rows).
Every core redundantly computes the full seq_fts (all j) plus f2; f1 only for
its own i-shard (via a separate per-core seq_shard input). The 2xNxN logits
are never materialized: a fused custom DVE op computes
d = lrelu(f1_0[i]+f2_0[j]) - lrelu(f1_1[i]+f2_1[j]) per [128j x 768i] tile,
ACT computes c0 = sigmoid(d), and the PE contracts c0 against
[fts0 | fts1] (float32r full-rate matmul). vals1 uses the complement trick:
vals1 = colsum(fts1) - c0 @ fts1.
"""

import numpy as np

import concourse.bacc as bacc
import concourse.bass as bass
import concourse.mybir as mybir
import concourse.tile as tile
from concourse.bass_utils import run_bass_kernel_spmd

B, N, H, O = 2, 6144, 256, 128
NCORES = 8
NS = N // NCORES          # 768 i-rows per core
NJT = N // 128            # 48 j-tiles
NIC = NS // 128           # 6 i-chunks per core
FP32 = mybir.dt.float32
BF16 = mybir.dt.bfloat16
F32R = mybir.dt.float32r
AF = mybir.ActivationFunctionType
ALU = mybir.AluOpType

_DVE_OP_NAME = "DIFF_LRELU_ANT"

DEFAULT_CFG = dict(
    lag=6,              # stage B lags stage A; MUST stay > the shard_finish
                        # emission point (loop iter 4) or stage B reads f1bc
                        # before it is written (program-order RAW violation)
    bufs_seq=4,
    bufs_sT=4,
    bufs_d=5,
    bufs_c=5,
    bufs_psA=2,
    bufs_psF=2,
    fts_copy="dve",     # engine for psum->fts copies: dve | act | alt
    stage_a="pe",       # pe (PE transpose) | dmat (bf16 DMA transpose)
    seq_bf16=False,     # cast seq fp32->bf16 during the HBM load (gpsimd DMA)
    sT_b0_copy="dve",   # legacy, unused
    sT_copy="act",      # engine for the merged transpose-block copy: dve | act | split
    ablate_scores=False,    # stage B matmuls use a constant lhsT (skip d/sigmoid)
    ablate_transposes=False,  # skip PE transposes + copies (sT from dummy)
    ablate_dma=False,       # load only one seq tile
)


def _get_diff_lrelu_op():
    """Register (once) and return the fused custom DVE op:
    out = lrelu(in0 + s0) - lrelu(in1 + s1), slope imm2."""
    import concourse.dve_ops as dve_ops
    from concourse.dve_ops import OPS, DveOp

    for op in OPS:
        if op.name == _DVE_OP_NAME:
            return op

    from concourse.dve_spec import C0, C1, C2, Spec, Src0, Src1, lower, maxx
    from concourse.dve_uop import DveOpSpec

    a = Src0 + C0
    b = Src1 + C1
    spec = Spec(
        body=maxx(a, a * C2) - maxx(b, b * C2),
        reference=lambda in0, in1, s0, s1, imm2: (
            np.maximum(in0 + s0, (in0 + s0) * imm2)
            - np.maximum(in1 + s1, (in1 + s1) * imm2)
        ).astype(np.float32),
    )
    row = dve_ops._CUSTOM_DVE_ROW_BASE + len(OPS)
    shas = {}
    for ver in ("v3",):
        uops = lower(spec, ver=ver)
        shas[ver] = DveOpSpec(
            name=_DVE_OP_NAME, opcode=row, uops=uops, rd1_en=True
        ).sha(ver)
    op = DveOp(_DVE_OP_NAME, spec, subdim=False, uops_sha=shas)
    OPS.append(op)
    dve_ops.CUSTOM_DVE_SPECS[_DVE_OP_NAME] = spec
    dve_ops._SUB_OPCODE_FOR_NAME[_DVE_OP_NAME] = row
    return op


def build_nc(probes=False, cfg=None):
    cfg = {**DEFAULT_CFG, **(cfg or {})}
    diff_lrelu = _get_diff_lrelu_op()

    nc = bacc.Bacc("TRN2", target_bir_lowering=False, debug=False, num_devices=NCORES)

    seq_d = nc.declare_dram_parameter("seq", [B, N, H], FP32, isOutput=False)
    seqs_d = nc.declare_dram_parameter("seq_shard", [B, NS, H], FP32, isOutput=False)
    wtg_d = nc.declare_dram_parameter("wtg", [2, 128, 256], FP32, isOutput=False)
    g1r_d = nc.declare_dram_parameter("g1r", [1, H], FP32, isOutput=False)
    consts_d = nc.declare_dram_parameter("consts", [1, 4], FP32, isOutput=False)
    ident_d = nc.declare_dram_parameter("ident", [128, 128], FP32, isOutput=False)
    out_d = nc.declare_dram_parameter("out", [B, NS, O], FP32, isOutput=True)
    if probes:
        pr_f1 = nc.declare_dram_parameter("pr_f1", [B, NS], FP32, isOutput=True)
        pr_f2 = nc.declare_dram_parameter("pr_f2", [B, NJT, 128], FP32, isOutput=True)
        pr_fts = nc.declare_dram_parameter("pr_fts", [B, NJT, 128, O], FP32, isOutput=True)
        pr_d = nc.declare_dram_parameter("pr_d", [128, NS], FP32, isOutput=True)
        pr_c0 = nc.declare_dram_parameter("pr_c0", [128, NS], FP32, isOutput=True)
        pr_s1 = nc.declare_dram_parameter("pr_s1", [1, O], FP32, isOutput=True)
        pr_tred = nc.declare_dram_parameter("pr_tred", [128, 2], FP32, isOutput=True)
        pr_vals = nc.declare_dram_parameter("pr_vals", [128, B, O], FP32, isOutput=True)

    with tile.TileContext(nc) as tc:
        with (
            tc.tile_pool(name="const", bufs=1) as cpool,
            tc.tile_pool(name="seq_in", bufs=cfg["bufs_seq"]) as p_seq,
            tc.tile_pool(name="sT", bufs=cfg["bufs_sT"]) as p_sT,
            tc.tile_pool(name="dtile", bufs=cfg["bufs_d"]) as p_d,
            tc.tile_pool(name="ctile", bufs=cfg["bufs_c"]) as p_c,
            tc.tile_pool(name="fin", bufs=8) as p_fin,
            tc.tile_pool(name="outt", bufs=4) as p_out,
        ):
            # ---------------- constants / setup ----------------
            wtg = cpool.tile([128, 2, 256], FP32)
            nc.sync.dma_start(wtg[:], wtg_d.ap().rearrange("k p c -> p k c"))
            wtg_r = cpool.tile([128, 2, 256], F32R)
            nc.vector.tensor_copy(wtg_r[:], wtg[:])
            wtg_bf = cpool.tile([128, 2, 256], BF16)
            nc.vector.tensor_copy(wtg_bf[:], wtg[:])
            ones_f = cpool.tile([128, 1], FP32)
            nc.gpsimd.memset(ones_f[:], 1.0)
            ones_r = cpool.tile([128, 1], F32R)
            nc.vector.tensor_copy(ones_r[:], ones_f[:])
            ident = cpool.tile([128, 128], FP32)
            nc.sync.dma_start(ident[:], ident_d[:])
            ident_bf = cpool.tile([128, 128], BF16)
            nc.vector.tensor_copy(ident_bf[:], ident[:])
            consts = cpool.tile([1, 4], FP32)
            nc.sync.dma_start(consts[:], consts_d[:])
            g1row = cpool.tile([1, H], FP32)
            nc.sync.dma_start(g1row[:], g1r_d[:])

            bias_col = cpool.tile([128, 1], FP32)
            nc.gpsimd.partition_broadcast(bias_col[:], consts[0:1, 1:2])
            biasm1_col = cpool.tile([128, 1], FP32)
            nc.gpsimd.partition_broadcast(biasm1_col[:], consts[0:1, 2:3])
            g1bc = cpool.tile([128, H], FP32)
            nc.gpsimd.partition_broadcast(g1bc[:], g1row[:])

            fts = cpool.tile([128, NJT, B, 130], F32R)     # [n128, jt, b, {fts[0:128], f1, f2}]
            f1row = [cpool.tile([1, NS], FP32, name=f"f1row{b}") for b in range(B)]
            f1bc = [cpool.tile([128, NS], FP32, name=f"f1bc{b}") for b in range(B)]
            s1bc = cpool.tile([128, O], FP32)
            s1row = cpool.tile([1, O], FP32)

            # shard pass pieces (f1 for own i-rows) - emitted interleaved below
            def shard_piece(k, ps_sh):
                del ps_sh
                b, nt = divmod(k, NIC)
                ss = p_seq.tile([128, H], FP32, name="ss", tag="s")
                nc.sync.dma_start(ss[:], seqs_d[b, nt * 128:(nt + 1) * 128, :])
                prod = p_sT.tile([128, H], FP32, tag="shard_prod")
                nc.vector.tensor_tensor(prod[:], ss[:], g1bc[:], ALU.mult)
                f1c = p_fin.tile([128, 1], FP32, tag="shard_f1c")
                nc.vector.tensor_reduce(f1c[:], prod[:], mybir.AxisListType.X, ALU.add)
                f1p = psA.tile([1, 128], FP32, name="f1p", tag="pt")
                nc.tensor.transpose(f1p[:], f1c[:], ident[:])
                nc.vector.tensor_copy(f1row[b][0:1, nt * 128:(nt + 1) * 128], f1p[:])

            def shard_finish():
                for b in range(B):
                    # += (f1_b + f2_b); the f2 column stays raw.
                    nc.vector.tensor_scalar(
                        f1row[b][:], f1row[b][:], consts[0:1, 0:1], None, ALU.add
                    )
                    nc.gpsimd.partition_broadcast(f1bc[b][:], f1row[b][:])

            # ------- stage A + stage B software-pipelined over j-tiles -------
            with (
                tc.tile_pool(name="psA", bufs=cfg["bufs_psA"], space="PSUM") as psA,
                tc.tile_pool(name="psF", bufs=cfg["bufs_psF"], space="PSUM") as psF,
                tc.tile_pool(name="psS", bufs=1, space="PSUM") as psS,
                tc.tile_pool(name="psB", bufs=1, space="PSUM") as psB,
            ):
                s1acc = psS.tile([1, B, O], FP32, name="s1acc", tag="s1acc")
                # 6 accumulators packed 2-per-bank: pacc_ap(ic) is one i-chunk
                pacc2 = [
                    psB.tile([128, 2, B, O], FP32, name=f"pacc{k}", tag=f"pacc{k}")
                    for k in range(NIC // 2)
                ]

                def pacc_ap(ic):
                    return pacc2[ic // 2][:, ic % 2]

                dummy_c = cpool.tile([128, NS], F32R)
                nc.vector.tensor_copy(dummy_c[:, 0:256], wtg_r[:, 0, :])
                if cfg["ablate_transposes"] or cfg["ablate_dma"]:
                    s0_tile = p_seq.tile([128, H], FP32, name="s", tag="s")
                    nc.sync.dma_start(s0_tile[:], seq_d[0, 0:128, :])
                    sT0 = p_sT.tile([128, 2, 128], F32R, name="sT0", tag="sT0")
                    for kt in range(2):
                        nc.vector.tensor_copy(sT0[:, kt, :], s0_tile[:, 0:128])
                if cfg["ablate_transposes"]:
                    nc.gpsimd.memset(tpart[:], 0.0)

                sT_tiles = {}

                def stage_t(jt):
                    # DMA in + PE transposes for j-tile jt; both batches in one
                    # DMA (dst [128 n, 2 b, 256 h], src strided over b)
                    bf = cfg["seq_bf16"]
                    sboth = p_seq.tile([128, B, H], BF16 if bf else FP32, name="s", tag="s")
                    src = seq_d[:, jt * 128:(jt + 1) * 128, :].rearrange("b n h -> n b h")
                    if bf:
                        nc.gpsimd.dma_start(sboth[:], src)
                    else:
                        nc.sync.dma_start(sboth[:], src)
                    ss = [sboth[:, b] for b in range(B)]
                    pt = psA.tile([128, 4, 128], BF16 if bf else FP32, name="pt", tag="pt")
                    sT = p_sT.tile([128, 4, 128], BF16 if bf else F32R, name="sT", tag="sT")
                    for b in range(B):
                        for kt in range(2):
                            nc.tensor.transpose(
                                pt[:, b * 2 + kt],
                                ss[b][:, kt * 128:(kt + 1) * 128],
                                ident_bf[:] if bf else ident[:],
                            )
                    if cfg["sT_copy"] == "dve":
                        nc.vector.tensor_copy(sT[:], pt[:])
                    elif cfg["sT_copy"] == "act":
                        nc.scalar.activation(sT[:], pt[:], AF.Copy)
                    else:
                        nc.vector.tensor_copy(sT[:, 0:2], pt[:, 0:2])
                        nc.scalar.activation(sT[:, 2:4], pt[:, 2:4], AF.Copy)
                    sT_tiles[jt] = sT

                def stage_m(jt):
                    # projection matmuls + psum->sbuf copy for j-tile jt
                    sT = sT_tiles.pop(jt)
                    fpp = psF.tile([128, 2, 256], FP32, name="fpp", tag="fpp")
                    for b in range(B):
                        for kt in range(2):
                            nc.tensor.matmul(
                                fpp[:, b],
                                lhsT=sT[:, b * 2 + kt, :],
                                rhs=(wtg_bf if cfg["seq_bf16"] else wtg_r)[:, kt, :],
                                start=(kt == 0), stop=(kt == 1),
                            )
                    eng = cfg["fts_copy"]
                    if eng == "alt":
                        eng = "dve" if jt % 2 == 0 else "act"
                    if eng == "dve":
                        nc.vector.tensor_copy(fts[:, jt, :, :], fpp[:, :, 0:130])
                    else:
                        nc.scalar.activation(fts[:, jt, :, :], fpp[:, :, 0:130], AF.Copy)

                def stage_b(jt):
                    if cfg["ablate_scores"]:
                        for ic in range(NIC):
                            nc.tensor.matmul(
                                pacc_ap(ic),
                                lhsT=dummy_c[:, ic * 128:(ic + 1) * 128],
                                rhs=fts[:, jt, :, 0:O],
                                start=(jt == 0),
                                stop=(jt == NJT - 1),
                            )
                        return
                    d = p_d.tile([128, NS], FP32, name="d", tag="d")
                    nc.vector._custom_dve(
                        diff_lrelu,
                        out=d[:],
                        in0=f1bc[0][:],
                        in1=f1bc[1][:],
                        s0=fts[:, jt, 0, 129:130].bitcast(FP32),
                        s1=fts[:, jt, 1, 129:130].bitcast(FP32),
                        imm2=0.01,
                    )
                    c0 = p_c.tile([128, NS], F32R, name="c0", tag="c0")
                    nc.scalar.activation(c0[:], d[:], AF.Sigmoid)
                    if probes and jt == 0:
                        nc.sync.dma_start(pr_d[:], d[:])
                        nc.sync.dma_start(pr_c0[:], c0[:].bitcast(FP32))
                    for ic in range(NIC):
                        # start=True clears the WHOLE psum bank, so only the
                        # first chunk sharing a bank may issue it; the second
                        # chunk's first write lands on cleared has_written
                        # bits and overwrites cleanly with start=False.
                        nc.tensor.matmul(
                            pacc_ap(ic),
                            lhsT=c0[:, ic * 128:(ic + 1) * 128],
                            rhs=fts[:, jt, :, 0:O],
                            start=(jt == 0 and ic % 2 == 0),
                            stop=(jt == NJT - 1),
                            skip_group_check=True,
                        )
                    nc.tensor.matmul(
                        s1acc[:], lhsT=ones_r[:], rhs=fts[:, jt, :, 0:O],
                        start=(jt == 0), stop=(jt == NJT - 1),
                    )

                lag = max(2, min(cfg["lag"], NJT))
                if True:
                    ps_sh = None
                    for jt in range(NJT + lag):
                        if jt < NJT:
                            stage_t(jt)
                        if jt < (2 * NIC) // 3:
                            for _k in range(3):
                                shard_piece(3 * jt + _k, ps_sh)
                        elif jt == (2 * NIC) // 3:
                            shard_finish()
                        if jt >= 1 and jt - 1 < NJT:
                            stage_m(jt - 1)
                        if jt >= lag:
                            stage_b(jt - lag)

                nc.vector.tensor_copy(s1row[:], s1acc[0:1, 1, :])
                nc.gpsimd.partition_broadcast(s1bc[:], s1row[:])
                if probes:
                    nc.sync.dma_start(pr_s1[:], s1row[:])
                    for b in range(B):
                        nc.sync.dma_start(pr_f1[b:b + 1, :], f1row[b][:])
                        for jt in range(NJT):
                            nc.sync.dma_start(
                                pr_f2[b, jt, :],
                                fts[:, jt, b, 129:130].bitcast(FP32),
                            )
                            nc.sync.dma_start(
                                pr_fts[b, jt, :, :],
                                fts[:, jt, b, 0:O].bitcast(FP32),
                            )
                    pv = p_c.tile([128, B * O], FP32, name="pv", tag="pv")
                    nc.vector.tensor_copy(pv[:], pacc_ap(0))
                    nc.sync.dma_start(pr_vals.ap().rearrange("p b o -> p (b o)"), pv[:])

                # ---------------- finalize: elu(vals + bias) ----------------
                def elu_store(src_ap, b, ic):
                    # elu(y) with y = src + bias: relu(y)-1 = max(y-1,-1); + exp(min(y,0))
                    r = p_fin.tile([128, O], FP32, tag="fin_r")
                    nc.vector.tensor_scalar(
                        r[:], src_ap, biasm1_col[:], -1.0, ALU.add, ALU.max
                    )
                    m = p_fin.tile([128, O], FP32, tag="fin_m")
                    nc.vector.tensor_scalar(
                        m[:], src_ap, bias_col[:], 0.0, ALU.add, ALU.min
                    )
                    e = p_fin.tile([128, O], FP32, tag="fin_e")
                    nc.scalar.activation(e[:], m[:], AF.Exp)
                    o = p_out.tile([128, O], FP32)
                    nc.vector.tensor_tensor(o[:], r[:], e[:], ALU.add)
                    nc.sync.dma_start(out_d[b, ic * 128:(ic + 1) * 128, :], o[:])

                for ic in range(NIC):
                    elu_store(pacc_ap(ic)[:, 0, :], 0, ic)
                    x1 = p_fin.tile([128, O], FP32, tag="fin_x1")
                    nc.vector.tensor_tensor(x1[:], s1bc[:], pacc_ap(ic)[:, 1, :], ALU.subtract)
                    elu_store(x1[:], 1, ic)

    nc.compile()
    return nc


def make_in_maps(seq, W_fts, f1_w, f1_b, f2_w, f2_b, bias):
    seq = np.ascontiguousarray(np.asarray(seq, dtype=np.float32))
    W = np.asarray(W_fts, dtype=np.float32)
    f1_w = np.asarray(f1_w, dtype=np.float32).reshape(-1)
    f2_w = np.asarray(f2_w, dtype=np.float32).reshape(-1)
    WT = np.ascontiguousarray(W.T)                      # [H, O]
    g1 = WT @ f1_w                                      # [H]
    g2 = WT @ f2_w
    wtg = np.zeros((2, 128, 256), np.float32)
    for kt in range(2):
        wtg[kt, :, 0:O] = WT[kt * 128:(kt + 1) * 128]
        wtg[kt, :, O] = g1[kt * 128:(kt + 1) * 128]
        wtg[kt, :, O + 1] = g2[kt * 128:(kt + 1) * 128]
    fsum = float(np.asarray(f1_b).reshape(-1)[0] + np.asarray(f2_b).reshape(-1)[0])
    bs = float(np.asarray(bias).reshape(-1)[0])
    consts = np.array([[fsum, bs, bs - 1.0, 0.0]], np.float32)
    ident = np.eye(128, dtype=np.float32)
    g1r = g1.reshape(1, H)

    in_maps = []
    for c in range(NCORES):
        in_maps.append({
            "seq": seq,
            "seq_shard": np.ascontiguousarray(seq[:, c * NS:(c + 1) * NS, :]),
            "wtg": wtg,
            "g1r": g1r,
            "consts": consts,
            "ident": ident,
        })
    return in_maps


_NC_CACHE = []


def kernel(seq, W_fts, f1_w, f1_b, f2_w, f2_b, bias):
    if not _NC_CACHE:
        _NC_CACHE.append(build_nc())
    nc = _NC_CACHE[0]
    in_maps = make_in_maps(seq, W_fts, f1_w, f1_b, f2_w, f2_b, bias)
    res = run_bass_kernel_spmd(nc, in_maps, core_ids=list(range(NCORES)))
    return np.concatenate([res.results[c]["out"] for c in range(NCORES)], axis=1)


# revision 35
# speedup vs baseline: 917.1714x; 1.0107x over previous
"""Trainium2 Bass kernel for a GAT-style attention head (B=2, N=6144, H=256, O=128).

Math (matching the reference):
  seq_fts = seq @ W_fts.T                       [B, N, O]
  f1 = seq_fts @ f1_w + f1_b                    [B, N]
  f2 = seq_fts @ f2_w + f2_b                    [B, N]
  z[b, j, i]  = leaky_relu(f1[b, i] + f2[b, j], 0.01)
  coefs[b,j,i] = softmax_b(z)   (B=2 -> coefs[0] = sigmoid(z0 - z1), coefs[1] = 1 - coefs[0])
  vals[b, i, o] = sum_j coefs[b, j, i] * seq_fts[b, j, o]
  out = elu(vals + bias)

Strategy: shard the output-row dim i across 8 cores (each core owns 768 rows).
Every core redundantly computes the full seq_fts (all j) plus f2; f1 only for
its own i-shard (via a separate per-core seq_shard input). The 2xNxN logits
are never materialized: a fused custom DVE op computes
d = lrelu(f1_0[i]+f2_0[j]) - lrelu(f1_1[i]+f2_1[j]) per [128j x 768i] tile,
ACT computes c0 = sigmoid(d), and the PE contracts c0 against
[fts0 | fts1] (float32r full-rate matmul). vals1 uses the complement trick:
vals1 = colsum(fts1) - c0 @ fts1.
"""

import numpy as np

import concourse.bacc as bacc
import concourse.bass as bass
import concourse.mybir as mybir
import concourse.tile as tile
from concourse.bass_utils import run_bass_kernel_spmd

B, N, H, O = 2, 6144, 256, 128
NCORES = 8
NS = N // NCORES          # 768 i-rows per core
NJT = N // 128            # 48 j-tiles
NIC = NS // 128           # 6 i-chunks per core
FP32 = mybir.dt.float32
BF16 = mybir.dt.bfloat16
F32R = mybir.dt.float32r
AF = mybir.ActivationFunctionType
ALU = mybir.AluOpType

_DVE_OP_NAME = "DIFF_LRELU_ANT"

DEFAULT_CFG = dict(
    lag=6,              # stage B lags stage A; MUST stay > the shard_finish
                        # emission point (loop iter 4) or stage B reads f1bc
                        # before it is written (program-order RAW violation)
    bufs_seq=4,
    bufs_sT=4,
    bufs_d=5,
    bufs_c=5,
    bufs_psA=2,
    bufs_psF=2,
    fts_copy="dve",     # engine for psum->fts copies: dve | act | alt
    stage_a="pe",       # pe (PE transpose) | dmat (bf16 DMA transpose)
    seq_bf16=False,     # cast seq fp32->bf16 during the HBM load (gpsimd DMA)
    f32r_transpose=False,  # PE transposes at f32r rate (1.5 vs 2.0 cyc/row)
    fin_add_gpsimd=True,   # finalize r+E adds on the idle GPSIMD engine
    sT_b0_copy="dve",   # legacy, unused
    sT_copy="act",      # engine for the merged transpose-block copy: dve | act | split
    ablate_scores=False,    # stage B matmuls use a constant lhsT (skip d/sigmoid)
    ablate_transposes=False,  # skip PE transposes + copies (sT from dummy)
    ablate_dma=False,       # load only one seq tile
)


def _get_diff_lrelu_op():
    """Register (once) and return the fused custom DVE op:
    out = lrelu(in0 + s0) - lrelu(in1 + s1), slope imm2."""
    import concourse.dve_ops as dve_ops
    from concourse.dve_ops import OPS, DveOp

    for op in OPS:
        if op.name == _DVE_OP_NAME:
            return op

    from concourse.dve_spec import C0, C1, C2, Spec, Src0, Src1, lower, maxx
    from concourse.dve_uop import DveOpSpec

    a = Src0 + C0
    b = Src1 + C1
    spec = Spec(
        body=maxx(a, a * C2) - maxx(b, b * C2),
        reference=lambda in0, in1, s0, s1, imm2: (
            np.maximum(in0 + s0, (in0 + s0) * imm2)
            - np.maximum(in1 + s1, (in1 + s1) * imm2)
        ).astype(np.float32),
    )
    row = dve_ops._CUSTOM_DVE_ROW_BASE + len(OPS)
    shas = {}
    for ver in ("v3",):
        uops = lower(spec, ver=ver)
        shas[ver] = DveOpSpec(
            name=_DVE_OP_NAME, opcode=row, uops=uops, rd1_en=True
        ).sha(ver)
    op = DveOp(_DVE_OP_NAME, spec, subdim=False, uops_sha=shas)
    OPS.append(op)
    dve_ops.CUSTOM_DVE_SPECS[_DVE_OP_NAME] = spec
    dve_ops._SUB_OPCODE_FOR_NAME[_DVE_OP_NAME] = row
    return op


def build_nc(probes=False, cfg=None):
    cfg = {**DEFAULT_CFG, **(cfg or {})}
    diff_lrelu = _get_diff_lrelu_op()

    nc = bacc.Bacc("TRN2", target_bir_lowering=False, debug=False, num_devices=NCORES)

    seq_d = nc.declare_dram_parameter("seq", [B, N, H], FP32, isOutput=False)
    seqs_d = nc.declare_dram_parameter("seq_shard", [B, NS, H], FP32, isOutput=False)
    wtg_d = nc.declare_dram_parameter("wtg", [2, 128, 256], FP32, isOutput=False)
    g1r_d = nc.declare_dram_parameter("g1r", [1, H], FP32, isOutput=False)
    consts_d = nc.declare_dram_parameter("consts", [1, 4], FP32, isOutput=False)
    ident_d = nc.declare_dram_parameter("ident", [128, 128], FP32, isOutput=False)
    out_d = nc.declare_dram_parameter("out", [B, NS, O], FP32, isOutput=True)
    if probes:
        pr_f1 = nc.declare_dram_parameter("pr_f1", [B, NS], FP32, isOutput=True)
        pr_f2 = nc.declare_dram_parameter("pr_f2", [B, NJT, 128], FP32, isOutput=True)
        pr_fts = nc.declare_dram_parameter("pr_fts", [B, NJT, 128, O], FP32, isOutput=True)
        pr_d = nc.declare_dram_parameter("pr_d", [128, NS], FP32, isOutput=True)
        pr_c0 = nc.declare_dram_parameter("pr_c0", [128, NS], FP32, isOutput=True)
        pr_s1 = nc.declare_dram_parameter("pr_s1", [1, O], FP32, isOutput=True)
        pr_tred = nc.declare_dram_parameter("pr_tred", [128, 2], FP32, isOutput=True)
        pr_vals = nc.declare_dram_parameter("pr_vals", [128, B, O], FP32, isOutput=True)

    with tile.TileContext(nc) as tc:
        with (
            tc.tile_pool(name="const", bufs=1) as cpool,
            tc.tile_pool(name="seq_in", bufs=cfg["bufs_seq"]) as p_seq,
            tc.tile_pool(name="sT", bufs=cfg["bufs_sT"]) as p_sT,
            tc.tile_pool(name="dtile", bufs=cfg["bufs_d"]) as p_d,
            tc.tile_pool(name="ctile", bufs=cfg["bufs_c"]) as p_c,
            tc.tile_pool(name="fin", bufs=8) as p_fin,
            tc.tile_pool(name="outt", bufs=4) as p_out,
        ):
            # ---------------- constants / setup ----------------
            wtg = cpool.tile([128, 2, 256], FP32)
            nc.sync.dma_start(wtg[:], wtg_d.ap().rearrange("k p c -> p k c"))
            wtg_r = cpool.tile([128, 2, 256], F32R)
            nc.vector.tensor_copy(wtg_r[:], wtg[:])
            wtg_bf = cpool.tile([128, 2, 256], BF16)
            nc.vector.tensor_copy(wtg_bf[:], wtg[:])
            ones_f = cpool.tile([128, 1], FP32)
            nc.gpsimd.memset(ones_f[:], 1.0)
            ones_r = cpool.tile([128, 1], F32R)
            nc.vector.tensor_copy(ones_r[:], ones_f[:])
            ident = cpool.tile([128, 128], FP32)
            nc.sync.dma_start(ident[:], ident_d[:])
            ident_bf = cpool.tile([128, 128], BF16)
            nc.vector.tensor_copy(ident_bf[:], ident[:])
            ident_r = cpool.tile([128, 128], F32R)
            nc.vector.tensor_copy(ident_r[:], ident[:])
            consts = cpool.tile([1, 4], FP32)
            nc.sync.dma_start(consts[:], consts_d[:])
            g1row = cpool.tile([1, H], FP32)
            nc.sync.dma_start(g1row[:], g1r_d[:])

            bias_col = cpool.tile([128, 1], FP32)
            nc.gpsimd.partition_broadcast(bias_col[:], consts[0:1, 1:2])
            biasm1_col = cpool.tile([128, 1], FP32)
            nc.gpsimd.partition_broadcast(biasm1_col[:], consts[0:1, 2:3])
            g1bc = cpool.tile([128, H], FP32)
            nc.gpsimd.partition_broadcast(g1bc[:], g1row[:])

            fts = cpool.tile([128, NJT, B, 130], F32R)     # [n128, jt, b, {fts[0:128], f1, f2}]
            f1row = [cpool.tile([1, NS], FP32, name=f"f1row{b}") for b in range(B)]
            f1bc = [cpool.tile([128, NS], FP32, name=f"f1bc{b}") for b in range(B)]
            s1bc = cpool.tile([128, O], FP32)
            s1row = cpool.tile([1, O], FP32)

            # shard pass pieces (f1 for own i-rows) - emitted interleaved below
            def shard_piece(k, ps_sh):
                del ps_sh
                b, nt = divmod(k, NIC)
                ss = p_seq.tile([128, H], FP32, name="ss", tag="s")
                nc.sync.dma_start(ss[:], seqs_d[b, nt * 128:(nt + 1) * 128, :])
                prod = p_sT.tile([128, H], FP32, tag="shard_prod")
                nc.vector.tensor_tensor(prod[:], ss[:], g1bc[:], ALU.mult)
                f1c = p_fin.tile([128, 1], FP32, tag="shard_f1c")
                nc.vector.tensor_reduce(f1c[:], prod[:], mybir.AxisListType.X, ALU.add)
                f1p = psA.tile([1, 128], FP32, name="f1p", tag="pt")
                nc.tensor.transpose(f1p[:], f1c[:], ident[:])
                nc.vector.tensor_copy(f1row[b][0:1, nt * 128:(nt + 1) * 128], f1p[:])

            def shard_finish():
                for b in range(B):
                    # += (f1_b + f2_b); the f2 column stays raw.
                    nc.vector.tensor_scalar(
                        f1row[b][:], f1row[b][:], consts[0:1, 0:1], None, ALU.add
                    )
                    nc.gpsimd.partition_broadcast(f1bc[b][:], f1row[b][:])

            # ------- stage A + stage B software-pipelined over j-tiles -------
            with (
                tc.tile_pool(name="psA", bufs=cfg["bufs_psA"], space="PSUM") as psA,
                tc.tile_pool(name="psF", bufs=cfg["bufs_psF"], space="PSUM") as psF,
                tc.tile_pool(name="psS", bufs=1, space="PSUM") as psS,
                tc.tile_pool(name="psB", bufs=1, space="PSUM") as psB,
            ):
                s1acc = psS.tile([1, B, O], FP32, name="s1acc", tag="s1acc")
                # 6 accumulators packed 2-per-bank: pacc_ap(ic) is one i-chunk
                pacc2 = [
                    psB.tile([128, 2, B, O], FP32, name=f"pacc{k}", tag=f"pacc{k}")
                    for k in range(NIC // 2)
                ]

                def pacc_ap(ic):
                    return pacc2[ic // 2][:, ic % 2]

                dummy_c = cpool.tile([128, NS], F32R)
                nc.vector.tensor_copy(dummy_c[:, 0:256], wtg_r[:, 0, :])
                if cfg["ablate_transposes"] or cfg["ablate_dma"]:
                    s0_tile = p_seq.tile([128, H], FP32, name="s", tag="s")
                    nc.sync.dma_start(s0_tile[:], seq_d[0, 0:128, :])
                    sT0 = p_sT.tile([128, 2, 128], F32R, name="sT0", tag="sT0")
                    for kt in range(2):
                        nc.vector.tensor_copy(sT0[:, kt, :], s0_tile[:, 0:128])
                if cfg["ablate_transposes"]:
                    nc.gpsimd.memset(tpart[:], 0.0)

                sT_tiles = {}

                def stage_t(jt):
                    # DMA in + PE transposes for j-tile jt; both batches in one
                    # DMA (dst [128 n, 2 b, 256 h], src strided over b)
                    bf = cfg["seq_bf16"]
                    sboth = p_seq.tile([128, B, H], BF16 if bf else FP32, name="s", tag="s")
                    src = seq_d[:, jt * 128:(jt + 1) * 128, :].rearrange("b n h -> n b h")
                    if bf:
                        nc.gpsimd.dma_start(sboth[:], src)
                    else:
                        nc.sync.dma_start(sboth[:], src)
                    ss = [sboth[:, b] for b in range(B)]
                    fr = cfg["f32r_transpose"] and not bf
                    pt = psA.tile([128, 4, 128], BF16 if bf else (F32R if fr else FP32),
                                  name="pt", tag="pt")
                    sT = p_sT.tile([128, 4, 128], BF16 if bf else F32R, name="sT", tag="sT")
                    for b in range(B):
                        for kt in range(2):
                            src = ss[b][:, kt * 128:(kt + 1) * 128]
                            nc.tensor.transpose(
                                pt[:, b * 2 + kt],
                                src.bitcast(F32R) if fr else src,
                                ident_bf[:] if bf else (ident_r[:] if fr else ident[:]),
                            )
                    if cfg["sT_copy"] == "dve":
                        nc.vector.tensor_copy(sT[:], pt[:])
                    elif cfg["sT_copy"] == "act":
                        nc.scalar.activation(sT[:], pt[:], AF.Copy)
                    else:
                        nc.vector.tensor_copy(sT[:, 0:2], pt[:, 0:2])
                        nc.scalar.activation(sT[:, 2:4], pt[:, 2:4], AF.Copy)
                    sT_tiles[jt] = sT

                def stage_m(jt):
                    # projection matmuls + psum->sbuf copy for j-tile jt
                    sT = sT_tiles.pop(jt)
                    fpp = psF.tile([128, 2, 256], FP32, name="fpp", tag="fpp")
                    for b in range(B):
                        for kt in range(2):
                            nc.tensor.matmul(
                                fpp[:, b],
                                lhsT=sT[:, b * 2 + kt, :],
                                rhs=(wtg_bf if cfg["seq_bf16"] else wtg_r)[:, kt, :],
                                start=(kt == 0), stop=(kt == 1),
                            )
                    eng = cfg["fts_copy"]
                    if eng == "alt":
                        eng = "dve" if jt % 2 == 0 else "act"
                    if eng == "dve":
                        nc.vector.tensor_copy(fts[:, jt, :, :], fpp[:, :, 0:130])
                    else:
                        nc.scalar.activation(fts[:, jt, :, :], fpp[:, :, 0:130], AF.Copy)

                def stage_b(jt):
                    if cfg["ablate_scores"]:
                        for ic in range(NIC):
                            nc.tensor.matmul(
                                pacc_ap(ic),
                                lhsT=dummy_c[:, ic * 128:(ic + 1) * 128],
                                rhs=fts[:, jt, :, 0:O],
                                start=(jt == 0),
                                stop=(jt == NJT - 1),
                            )
                        return
                    d = p_d.tile([128, NS], FP32, name="d", tag="d")
                    nc.vector._custom_dve(
                        diff_lrelu,
                        out=d[:],
                        in0=f1bc[0][:],
                        in1=f1bc[1][:],
                        s0=fts[:, jt, 0, 129:130].bitcast(FP32),
                        s1=fts[:, jt, 1, 129:130].bitcast(FP32),
                        imm2=0.01,
                    )
                    c0 = p_c.tile([128, NS], F32R, name="c0", tag="c0")
                    nc.scalar.activation(c0[:], d[:], AF.Sigmoid)
                    if probes and jt == 0:
                        nc.sync.dma_start(pr_d[:], d[:])
                        nc.sync.dma_start(pr_c0[:], c0[:].bitcast(FP32))
                    for ic in range(NIC):
                        # start=True clears the WHOLE psum bank, so only the
                        # first chunk sharing a bank may issue it; the second
                        # chunk's first write lands on cleared has_written
                        # bits and overwrites cleanly with start=False.
                        nc.tensor.matmul(
                            pacc_ap(ic),
                            lhsT=c0[:, ic * 128:(ic + 1) * 128],
                            rhs=fts[:, jt, :, 0:O],
                            start=(jt == 0 and ic % 2 == 0),
                            stop=(jt == NJT - 1),
                            skip_group_check=True,
                        )
                    nc.tensor.matmul(
                        s1acc[:], lhsT=ones_r[:], rhs=fts[:, jt, :, 0:O],
                        start=(jt == 0), stop=(jt == NJT - 1),
                    )

                lag = max(2, min(cfg["lag"], NJT))
                if True:
                    ps_sh = None
                    for jt in range(NJT + lag):
                        if jt < NJT:
                            stage_t(jt)
                        if jt < (2 * NIC) // 3:
                            for _k in range(3):
                                shard_piece(3 * jt + _k, ps_sh)
                        elif jt == (2 * NIC) // 3:
                            shard_finish()
                        if jt >= 1 and jt - 1 < NJT:
                            stage_m(jt - 1)
                        if jt >= lag:
                            stage_b(jt - lag)

                nc.vector.tensor_copy(s1row[:], s1acc[0:1, 1, :])
                nc.gpsimd.partition_broadcast(s1bc[:], s1row[:])
                if probes:
                    nc.sync.dma_start(pr_s1[:], s1row[:])
                    for b in range(B):
                        nc.sync.dma_start(pr_f1[b:b + 1, :], f1row[b][:])
                        for jt in range(NJT):
                            nc.sync.dma_start(
                                pr_f2[b, jt, :],
                                fts[:, jt, b, 129:130].bitcast(FP32),
                            )
                            nc.sync.dma_start(
                                pr_fts[b, jt, :, :],
                                fts[:, jt, b, 0:O].bitcast(FP32),
                            )
                    pv = p_c.tile([128, B * O], FP32, name="pv", tag="pv")
                    nc.vector.tensor_copy(pv[:], pacc_ap(0))
                    nc.sync.dma_start(pr_vals.ap().rearrange("p b o -> p (b o)"), pv[:])

                # ---------------- finalize: elu(vals + bias) ----------------
                def elu_store(src_ap, b, ic):
                    # elu(y) with y = src + bias: relu(y)-1 = max(y-1,-1); + exp(min(y,0))
                    r = p_fin.tile([128, O], FP32, tag="fin_r")
                    nc.vector.tensor_scalar(
                        r[:], src_ap, biasm1_col[:], -1.0, ALU.add, ALU.max
                    )
                    m = p_fin.tile([128, O], FP32, tag="fin_m")
                    nc.vector.tensor_scalar(
                        m[:], src_ap, bias_col[:], 0.0, ALU.add, ALU.min
                    )
                    e = p_fin.tile([128, O], FP32, tag="fin_e")
                    nc.scalar.activation(e[:], m[:], AF.Exp)
                    o = p_out.tile([128, O], FP32)
                    if cfg["fin_add_gpsimd"]:
                        nc.gpsimd.tensor_tensor(o[:], r[:], e[:], ALU.add)
                    else:
                        nc.vector.tensor_tensor(o[:], r[:], e[:], ALU.add)
                    nc.sync.dma_start(out_d[b, ic * 128:(ic + 1) * 128, :], o[:])

                for ic in range(NIC):
                    elu_store(pacc_ap(ic)[:, 0, :], 0, ic)
                    x1 = p_fin.tile([128, O], FP32, tag="fin_x1")
                    nc.vector.tensor_tensor(x1[:], s1bc[:], pacc_ap(ic)[:, 1, :], ALU.subtract)
                    elu_store(x1[:], 1, ic)

    nc.compile()
    return nc


def make_in_maps(seq, W_fts, f1_w, f1_b, f2_w, f2_b, bias):
    seq = np.ascontiguousarray(np.asarray(seq, dtype=np.float32))
    W = np.asarray(W_fts, dtype=np.float32)
    f1_w = np.asarray(f1_w, dtype=np.float32).reshape(-1)
    f2_w = np.asarray(f2_w, dtype=np.float32).reshape(-1)
    WT = np.ascontiguousarray(W.T)                      # [H, O]
    g1 = WT @ f1_w                                      # [H]
    g2 = WT @ f2_w
    wtg = np.zeros((2, 128, 256), np.float32)
    for kt in range(2):
        wtg[kt, :, 0:O] = WT[kt * 128:(kt + 1) * 128]
        wtg[kt, :, O] = g1[kt * 128:(kt + 1) * 128]
        wtg[kt, :, O + 1] = g2[kt * 128:(kt + 1) * 128]
    fsum = float(np.asarray(f1_b).reshape(-1)[0] + np.asarray(f2_b).reshape(-1)[0])
    bs = float(np.asarray(bias).reshape(-1)[0])
    consts = np.array([[fsum, bs, bs - 1.0, 0.0]], np.float32)
    ident = np.eye(128, dtype=np.float32)
    g1r = g1.reshape(1, H)

    in_maps = []
    for c in range(NCORES):
        in_maps.append({
            "seq": seq,
            "seq_shard": np.ascontiguousarray(seq[:, c * NS:(c + 1) * NS, :]),
            "wtg": wtg,
            "g1r": g1r,
            "consts": consts,
            "ident": ident,
        })
    return in_maps


_NC_CACHE = []


def kernel(seq, W_fts, f1_w, f1_b, f2_w, f2_b, bias):
    if not _NC_CACHE:
        _NC_CACHE.append(build_nc())
    nc = _NC_CACHE[0]
    in_maps = make_in_maps(seq, W_fts, f1_w, f1_b, f2_w, f2_b, bias)
    res = run_bass_kernel_spmd(nc, in_maps, core_ids=list(range(NCORES)))
    return np.concatenate([res.results[c]["out"] for c in range(NCORES)], axis=1)


# revision 36
# speedup vs baseline: 935.4477x; 1.0199x over previous
"""Trainium2 Bass kernel for a GAT-style attention head (B=2, N=6144, H=256, O=128).

Math (matching the reference):
  seq_fts = seq @ W_fts.T                       [B, N, O]
  f1 = seq_fts @ f1_w + f1_b                    [B, N]
  f2 = seq_fts @ f2_w + f2_b                    [B, N]
  z[b, j, i]  = leaky_relu(f1[b, i] + f2[b, j], 0.01)
  coefs[b,j,i] = softmax_b(z)   (B=2 -> coefs[0] = sigmoid(z0 - z1), coefs[1] = 1 - coefs[0])
  vals[b, i, o] = sum_j coefs[b, j, i] * seq_fts[b, j, o]
  out = elu(vals + bias)

Strategy: shard the output-row dim i across 8 cores (each core owns 768 rows).
Every core redundantly computes the full seq_fts (all j) plus f2; f1 only for
its own i-shard (via a separate per-core seq_shard input). The 2xNxN logits
are never materialized: a fused custom DVE op computes
d = lrelu(f1_0[i]+f2_0[j]) - lrelu(f1_1[i]+f2_1[j]) per [128j x 768i] tile,
ACT computes c0 = sigmoid(d), and the PE contracts c0 against
[fts0 | fts1] (float32r full-rate matmul). vals1 uses the complement trick:
vals1 = colsum(fts1) - c0 @ fts1.
"""

import numpy as np

import concourse.bacc as bacc
import concourse.bass as bass
import concourse.mybir as mybir
import concourse.tile as tile
from concourse.bass_utils import run_bass_kernel_spmd

B, N, H, O = 2, 6144, 256, 128
NCORES = 8
NS = N // NCORES          # 768 i-rows per core
NJT = N // 128            # 48 j-tiles
NIC = NS // 128           # 6 i-chunks per core
FP32 = mybir.dt.float32
BF16 = mybir.dt.bfloat16
F32R = mybir.dt.float32r
AF = mybir.ActivationFunctionType
ALU = mybir.AluOpType

_DVE_OP_NAME = "DIFF_LRELU_ANT"

DEFAULT_CFG = dict(
    lag=6,              # stage B lags stage A; MUST stay > the shard_finish
                        # emission point (loop iter 4) or stage B reads f1bc
                        # before it is written (program-order RAW violation)
    bufs_seq=6,
    bufs_sT=6,
    bufs_d=5,
    bufs_c=5,
    bufs_psA=2,
    bufs_psF=2,
    fts_copy="dve",     # engine for psum->fts copies: dve | act | alt
    stage_a="pe",       # pe (PE transpose) | dmat (bf16 DMA transpose)
    seq_bf16=False,     # cast seq fp32->bf16 during the HBM load (gpsimd DMA)
    f32r_transpose=False,  # PE transposes at f32r rate (1.5 vs 2.0 cyc/row)
    fin_add_gpsimd=True,   # finalize r+E adds on the idle GPSIMD engine
    sT_b0_copy="dve",   # legacy, unused
    sT_copy="act",      # engine for the merged transpose-block copy: dve | act | split
    ablate_scores=False,    # stage B matmuls use a constant lhsT (skip d/sigmoid)
    ablate_transposes=False,  # skip PE transposes + copies (sT from dummy)
    ablate_dma=False,       # load only one seq tile
)


def _get_diff_lrelu_op():
    """Register (once) and return the fused custom DVE op:
    out = lrelu(in0 + s0) - lrelu(in1 + s1), slope imm2."""
    import concourse.dve_ops as dve_ops
    from concourse.dve_ops import OPS, DveOp

    for op in OPS:
        if op.name == _DVE_OP_NAME:
            return op

    from concourse.dve_spec import C0, C1, C2, Spec, Src0, Src1, lower, maxx
    from concourse.dve_uop import DveOpSpec

    a = Src0 + C0
    b = Src1 + C1
    spec = Spec(
        body=maxx(a, a * C2) - maxx(b, b * C2),
        reference=lambda in0, in1, s0, s1, imm2: (
            np.maximum(in0 + s0, (in0 + s0) * imm2)
            - np.maximum(in1 + s1, (in1 + s1) * imm2)
        ).astype(np.float32),
    )
    row = dve_ops._CUSTOM_DVE_ROW_BASE + len(OPS)
    shas = {}
    for ver in ("v3",):
        uops = lower(spec, ver=ver)
        shas[ver] = DveOpSpec(
            name=_DVE_OP_NAME, opcode=row, uops=uops, rd1_en=True
        ).sha(ver)
    op = DveOp(_DVE_OP_NAME, spec, subdim=False, uops_sha=shas)
    OPS.append(op)
    dve_ops.CUSTOM_DVE_SPECS[_DVE_OP_NAME] = spec
    dve_ops._SUB_OPCODE_FOR_NAME[_DVE_OP_NAME] = row
    return op


def build_nc(probes=False, cfg=None):
    cfg = {**DEFAULT_CFG, **(cfg or {})}
    diff_lrelu = _get_diff_lrelu_op()

    nc = bacc.Bacc("TRN2", target_bir_lowering=False, debug=False, num_devices=NCORES)

    seq_d = nc.declare_dram_parameter("seq", [B, N, H], FP32, isOutput=False)
    seqs_d = nc.declare_dram_parameter("seq_shard", [B, NS, H], FP32, isOutput=False)
    wtg_d = nc.declare_dram_parameter("wtg", [2, 128, 256], FP32, isOutput=False)
    g1r_d = nc.declare_dram_parameter("g1r", [1, H], FP32, isOutput=False)
    consts_d = nc.declare_dram_parameter("consts", [1, 4], FP32, isOutput=False)
    ident_d = nc.declare_dram_parameter("ident", [128, 128], FP32, isOutput=False)
    out_d = nc.declare_dram_parameter("out", [B, NS, O], FP32, isOutput=True)
    if probes:
        pr_f1 = nc.declare_dram_parameter("pr_f1", [B, NS], FP32, isOutput=True)
        pr_f2 = nc.declare_dram_parameter("pr_f2", [B, NJT, 128], FP32, isOutput=True)
        pr_fts = nc.declare_dram_parameter("pr_fts", [B, NJT, 128, O], FP32, isOutput=True)
        pr_d = nc.declare_dram_parameter("pr_d", [128, NS], FP32, isOutput=True)
        pr_c0 = nc.declare_dram_parameter("pr_c0", [128, NS], FP32, isOutput=True)
        pr_s1 = nc.declare_dram_parameter("pr_s1", [1, O], FP32, isOutput=True)
        pr_tred = nc.declare_dram_parameter("pr_tred", [128, 2], FP32, isOutput=True)
        pr_vals = nc.declare_dram_parameter("pr_vals", [128, B, O], FP32, isOutput=True)

    with tile.TileContext(nc) as tc:
        with (
            tc.tile_pool(name="const", bufs=1) as cpool,
            tc.tile_pool(name="seq_in", bufs=cfg["bufs_seq"]) as p_seq,
            tc.tile_pool(name="sT", bufs=cfg["bufs_sT"]) as p_sT,
            tc.tile_pool(name="dtile", bufs=cfg["bufs_d"]) as p_d,
            tc.tile_pool(name="ctile", bufs=cfg["bufs_c"]) as p_c,
            tc.tile_pool(name="fin", bufs=8) as p_fin,
            tc.tile_pool(name="outt", bufs=4) as p_out,
        ):
            # ---------------- constants / setup ----------------
            wtg = cpool.tile([128, 2, 256], FP32)
            nc.sync.dma_start(wtg[:], wtg_d.ap().rearrange("k p c -> p k c"))
            wtg_r = cpool.tile([128, 2, 256], F32R)
            nc.vector.tensor_copy(wtg_r[:], wtg[:])
            wtg_bf = cpool.tile([128, 2, 256], BF16)
            nc.vector.tensor_copy(wtg_bf[:], wtg[:])
            ones_f = cpool.tile([128, 1], FP32)
            nc.gpsimd.memset(ones_f[:], 1.0)
            ones_r = cpool.tile([128, 1], F32R)
            nc.vector.tensor_copy(ones_r[:], ones_f[:])
            ident = cpool.tile([128, 128], FP32)
            nc.sync.dma_start(ident[:], ident_d[:])
            ident_bf = cpool.tile([128, 128], BF16)
            nc.vector.tensor_copy(ident_bf[:], ident[:])
            ident_r = cpool.tile([128, 128], F32R)
            nc.vector.tensor_copy(ident_r[:], ident[:])
            consts = cpool.tile([1, 4], FP32)
            nc.sync.dma_start(consts[:], consts_d[:])
            g1row = cpool.tile([1, H], FP32)
            nc.sync.dma_start(g1row[:], g1r_d[:])

            bias_col = cpool.tile([128, 1], FP32)
            nc.gpsimd.partition_broadcast(bias_col[:], consts[0:1, 1:2])
            biasm1_col = cpool.tile([128, 1], FP32)
            nc.gpsimd.partition_broadcast(biasm1_col[:], consts[0:1, 2:3])
            g1bc = cpool.tile([128, H], FP32)
            nc.gpsimd.partition_broadcast(g1bc[:], g1row[:])

            fts = cpool.tile([128, NJT, B, 130], F32R)     # [n128, jt, b, {fts[0:128], f1, f2}]
            f1row = [cpool.tile([1, NS], FP32, name=f"f1row{b}") for b in range(B)]
            f1bc = [cpool.tile([128, NS], FP32, name=f"f1bc{b}") for b in range(B)]
            s1bc = cpool.tile([128, O], FP32)
            s1row = cpool.tile([1, O], FP32)

            # shard pass pieces (f1 for own i-rows) - emitted interleaved below
            def shard_piece(k, ps_sh):
                del ps_sh
                b, nt = divmod(k, NIC)
                ss = p_seq.tile([128, H], FP32, name="ss", tag="s")
                nc.sync.dma_start(ss[:], seqs_d[b, nt * 128:(nt + 1) * 128, :])
                prod = p_sT.tile([128, H], FP32, tag="shard_prod")
                nc.vector.tensor_tensor(prod[:], ss[:], g1bc[:], ALU.mult)
                f1c = p_fin.tile([128, 1], FP32, tag="shard_f1c")
                nc.vector.tensor_reduce(f1c[:], prod[:], mybir.AxisListType.X, ALU.add)
                f1p = psA.tile([1, 128], FP32, name="f1p", tag="pt")
                nc.tensor.transpose(f1p[:], f1c[:], ident[:])
                nc.vector.tensor_copy(f1row[b][0:1, nt * 128:(nt + 1) * 128], f1p[:])

            def shard_finish():
                for b in range(B):
                    # += (f1_b + f2_b); the f2 column stays raw.
                    nc.vector.tensor_scalar(
                        f1row[b][:], f1row[b][:], consts[0:1, 0:1], None, ALU.add
                    )
                    nc.gpsimd.partition_broadcast(f1bc[b][:], f1row[b][:])

            # ------- stage A + stage B software-pipelined over j-tiles -------
            with (
                tc.tile_pool(name="psA", bufs=cfg["bufs_psA"], space="PSUM") as psA,
                tc.tile_pool(name="psF", bufs=cfg["bufs_psF"], space="PSUM") as psF,
                tc.tile_pool(name="psS", bufs=1, space="PSUM") as psS,
                tc.tile_pool(name="psB", bufs=1, space="PSUM") as psB,
            ):
                s1acc = psS.tile([1, B, O], FP32, name="s1acc", tag="s1acc")
                # 6 accumulators packed 2-per-bank: pacc_ap(ic) is one i-chunk
                pacc2 = [
                    psB.tile([128, 2, B, O], FP32, name=f"pacc{k}", tag=f"pacc{k}")
                    for k in range(NIC // 2)
                ]

                def pacc_ap(ic):
                    return pacc2[ic // 2][:, ic % 2]

                dummy_c = cpool.tile([128, NS], F32R)
                nc.vector.tensor_copy(dummy_c[:, 0:256], wtg_r[:, 0, :])
                if cfg["ablate_transposes"] or cfg["ablate_dma"]:
                    s0_tile = p_seq.tile([128, H], FP32, name="s", tag="s")
                    nc.sync.dma_start(s0_tile[:], seq_d[0, 0:128, :])
                    sT0 = p_sT.tile([128, 2, 128], F32R, name="sT0", tag="sT0")
                    for kt in range(2):
                        nc.vector.tensor_copy(sT0[:, kt, :], s0_tile[:, 0:128])
                if cfg["ablate_transposes"]:
                    nc.gpsimd.memset(tpart[:], 0.0)

                sT_tiles = {}

                def stage_t(jt):
                    # DMA in + PE transposes for j-tile jt; both batches in one
                    # DMA (dst [128 n, 2 b, 256 h], src strided over b)
                    bf = cfg["seq_bf16"]
                    sboth = p_seq.tile([128, B, H], BF16 if bf else FP32, name="s", tag="s")
                    src = seq_d[:, jt * 128:(jt + 1) * 128, :].rearrange("b n h -> n b h")
                    if bf:
                        nc.gpsimd.dma_start(sboth[:], src)
                    else:
                        nc.sync.dma_start(sboth[:], src)
                    ss = [sboth[:, b] for b in range(B)]
                    fr = cfg["f32r_transpose"] and not bf
                    pt = psA.tile([128, 4, 128], BF16 if bf else (F32R if fr else FP32),
                                  name="pt", tag="pt")
                    sT = p_sT.tile([128, 4, 128], BF16 if bf else F32R, name="sT", tag="sT")
                    for b in range(B):
                        for kt in range(2):
                            src = ss[b][:, kt * 128:(kt + 1) * 128]
                            nc.tensor.transpose(
                                pt[:, b * 2 + kt],
                                src.bitcast(F32R) if fr else src,
                                ident_bf[:] if bf else (ident_r[:] if fr else ident[:]),
                            )
                    if cfg["sT_copy"] == "dve":
                        nc.vector.tensor_copy(sT[:], pt[:])
                    elif cfg["sT_copy"] == "act":
                        nc.scalar.activation(sT[:], pt[:], AF.Copy)
                    else:
                        nc.vector.tensor_copy(sT[:, 0:2], pt[:, 0:2])
                        nc.scalar.activation(sT[:, 2:4], pt[:, 2:4], AF.Copy)
                    sT_tiles[jt] = sT

                def stage_m(jt):
                    # projection matmuls + psum->sbuf copy for j-tile jt
                    sT = sT_tiles.pop(jt)
                    fpp = psF.tile([128, 2, 256], FP32, name="fpp", tag="fpp")
                    for b in range(B):
                        for kt in range(2):
                            nc.tensor.matmul(
                                fpp[:, b],
                                lhsT=sT[:, b * 2 + kt, :],
                                rhs=(wtg_bf if cfg["seq_bf16"] else wtg_r)[:, kt, :],
                                start=(kt == 0), stop=(kt == 1),
                            )
                    eng = cfg["fts_copy"]
                    if eng == "alt":
                        eng = "dve" if jt % 2 == 0 else "act"
                    if eng == "dve":
                        nc.vector.tensor_copy(fts[:, jt, :, :], fpp[:, :, 0:130])
                    else:
                        nc.scalar.activation(fts[:, jt, :, :], fpp[:, :, 0:130], AF.Copy)

                def stage_b(jt):
                    if cfg["ablate_scores"]:
                        for ic in range(NIC):
                            nc.tensor.matmul(
                                pacc_ap(ic),
                                lhsT=dummy_c[:, ic * 128:(ic + 1) * 128],
                                rhs=fts[:, jt, :, 0:O],
                                start=(jt == 0),
                                stop=(jt == NJT - 1),
                            )
                        return
                    d = p_d.tile([128, NS], FP32, name="d", tag="d")
                    nc.vector._custom_dve(
                        diff_lrelu,
                        out=d[:],
                        in0=f1bc[0][:],
                        in1=f1bc[1][:],
                        s0=fts[:, jt, 0, 129:130].bitcast(FP32),
                        s1=fts[:, jt, 1, 129:130].bitcast(FP32),
                        imm2=0.01,
                    )
                    c0 = p_c.tile([128, NS], F32R, name="c0", tag="c0")
                    nc.scalar.activation(c0[:], d[:], AF.Sigmoid)
                    if probes and jt == 0:
                        nc.sync.dma_start(pr_d[:], d[:])
                        nc.sync.dma_start(pr_c0[:], c0[:].bitcast(FP32))
                    for ic in range(NIC):
                        # start=True clears the WHOLE psum bank, so only the
                        # first chunk sharing a bank may issue it; the second
                        # chunk's first write lands on cleared has_written
                        # bits and overwrites cleanly with start=False.
                        nc.tensor.matmul(
                            pacc_ap(ic),
                            lhsT=c0[:, ic * 128:(ic + 1) * 128],
                            rhs=fts[:, jt, :, 0:O],
                            start=(jt == 0 and ic % 2 == 0),
                            stop=(jt == NJT - 1),
                            skip_group_check=True,
                        )
                    nc.tensor.matmul(
                        s1acc[:], lhsT=ones_r[:], rhs=fts[:, jt, :, 0:O],
                        start=(jt == 0), stop=(jt == NJT - 1),
                    )

                lag = max(2, min(cfg["lag"], NJT))
                if True:
                    ps_sh = None
                    for jt in range(NJT + lag):
                        if jt < NJT:
                            stage_t(jt)
                        if jt < (2 * NIC) // 3:
                            for _k in range(3):
                                shard_piece(3 * jt + _k, ps_sh)
                        elif jt == (2 * NIC) // 3:
                            shard_finish()
                        if jt >= 1 and jt - 1 < NJT:
                            stage_m(jt - 1)
                        if jt >= lag:
                            stage_b(jt - lag)

                nc.vector.tensor_copy(s1row[:], s1acc[0:1, 1, :])
                nc.gpsimd.partition_broadcast(s1bc[:], s1row[:])
                if probes:
                    nc.sync.dma_start(pr_s1[:], s1row[:])
                    for b in range(B):
                        nc.sync.dma_start(pr_f1[b:b + 1, :], f1row[b][:])
                        for jt in range(NJT):
                            nc.sync.dma_start(
                                pr_f2[b, jt, :],
                                fts[:, jt, b, 129:130].bitcast(FP32),
                            )
                            nc.sync.dma_start(
                                pr_fts[b, jt, :, :],
                                fts[:, jt, b, 0:O].bitcast(FP32),
                            )
                    pv = p_c.tile([128, B * O], FP32, name="pv", tag="pv")
                    nc.vector.tensor_copy(pv[:], pacc_ap(0))
                    nc.sync.dma_start(pr_vals.ap().rearrange("p b o -> p (b o)"), pv[:])

                # ---------------- finalize: elu(vals + bias) ----------------
                def elu_store(src_ap, b, ic):
                    # elu(y) with y = src + bias: relu(y)-1 = max(y-1,-1); + exp(min(y,0))
                    r = p_fin.tile([128, O], FP32, tag="fin_r")
                    nc.vector.tensor_scalar(
                        r[:], src_ap, biasm1_col[:], -1.0, ALU.add, ALU.max
                    )
                    m = p_fin.tile([128, O], FP32, tag="fin_m")
                    nc.vector.tensor_scalar(
                        m[:], src_ap, bias_col[:], 0.0, ALU.add, ALU.min
                    )
                    e = p_fin.tile([128, O], FP32, tag="fin_e")
                    nc.scalar.activation(e[:], m[:], AF.Exp)
                    o = p_out.tile([128, O], FP32)
                    if cfg["fin_add_gpsimd"]:
                        nc.gpsimd.tensor_tensor(o[:], r[:], e[:], ALU.add)
                    else:
                        nc.vector.tensor_tensor(o[:], r[:], e[:], ALU.add)
                    nc.sync.dma_start(out_d[b, ic * 128:(ic + 1) * 128, :], o[:])

                for ic in range(NIC):
                    elu_store(pacc_ap(ic)[:, 0, :], 0, ic)
                    x1 = p_fin.tile([128, O], FP32, tag="fin_x1")
                    nc.vector.tensor_tensor(x1[:], s1bc[:], pacc_ap(ic)[:, 1, :], ALU.subtract)
                    elu_store(x1[:], 1, ic)

    nc.compile()
    return nc


def make_in_maps(seq, W_fts, f1_w, f1_b, f2_w, f2_b, bias):
    seq = np.ascontiguousarray(np.asarray(seq, dtype=np.float32))
    W = np.asarray(W_fts, dtype=np.float32)
    f1_w = np.asarray(f1_w, dtype=np.float32).reshape(-1)
    f2_w = np.asarray(f2_w, dtype=np.float32).reshape(-1)
    WT = np.ascontiguousarray(W.T)                      # [H, O]
    g1 = WT @ f1_w                                      # [H]
    g2 = WT @ f2_w
    wtg = np.zeros((2, 128, 256), np.float32)
    for kt in range(2):
        wtg[kt, :, 0:O] = WT[kt * 128:(kt + 1) * 128]
        wtg[kt, :, O] = g1[kt * 128:(kt + 1) * 128]
        wtg[kt, :, O + 1] = g2[kt * 128:(kt + 1) * 128]
    fsum = float(np.asarray(f1_b).reshape(-1)[0] + np.asarray(f2_b).reshape(-1)[0])
    bs = float(np.asarray(bias).reshape(-1)[0])
    consts = np.array([[fsum, bs, bs - 1.0, 0.0]], np.float32)
    ident = np.eye(128, dtype=np.float32)
    g1r = g1.reshape(1, H)

    in_maps = []
    for c in range(NCORES):
        in_maps.append({
            "seq": seq,
            "seq_shard": np.ascontiguousarray(seq[:, c * NS:(c + 1) * NS, :]),
            "wtg": wtg,
            "g1r": g1r,
            "consts": consts,
            "ident": ident,
        })
    return in_maps


_NC_CACHE = []


def kernel(seq, W_fts, f1_w, f1_b, f2_w, f2_b, bias):
    if not _NC_CACHE:
        _NC_CACHE.append(build_nc())
    nc = _NC_CACHE[0]
    in_maps = make_in_maps(seq, W_fts, f1_w, f1_b, f2_w, f2_b, bias)
    res = run_bass_kernel_spmd(nc, in_maps, core_ids=list(range(NCORES)))
    return np.concatenate([res.results[c]["out"] for c in range(NCORES)], axis=1)
